# revision 3
# baseline (speedup 1.0000x reference)
"""Trainium2 Bass kernel for a dense pre-norm transformer block (fp8 version).

Problem: x[8, 1024, 768]; per-batch-element transformer block
  (LN1 -> qkv -> 12-head attention -> proj residual -> LN2 -> MLP(gelu) residual).

Strategy (v2):
  - Pure data-parallel: 8 NeuronCores, one batch element each. No collectives.
  - Channel-major activations ([C, tokens]) on device; host transposes.
  - All big GEMMs (q/k/v production, attn@V, proj, fc1, fc2) run fp8-E4M3
    with perf_mode=DoubleRow: 2 contraction tiles per matmul, ~2x PE rate and
    half the weight DMA traffic.  Scores (K=64) stay bf16, packed two heads
    per PE pass via row groups.  PSUM accumulation is fp32 throughout; the
    residual stream and LN statistics stay fp32.
  - LayerNorm stats via ones-matmuls on bitcast f32r (no copies); squares on
    GpSimd; normalize split DVE/GpSimd; final scale-bias on DVE tensor_scalar.
  - All 12 q/k matrices are produced up-front after LN1 so the per-pair
    exp (ScalarE) pipeline runs without PE-side psum contention.
  - exp consumes score PSUM directly and writes fp8 tiles shaped [P, 2, N]
    (two key tiles) which feed DoubleRow attn@V directly; softmax denominators
    ride in a ones-column appended to V (slot padded to 80 bytes for the DR
    16B-stride rule); per-head normalization is broadcast across partitions
    via a small DRAM round-trip, pipelined per head-pair.
  - Weights stream per rep as a handful of large HWDGE DMAs.
"""

import ml_dtypes
import numpy as np

import concourse.bacc as bacc
import concourse.bass as bass
import concourse.mybir as mybir
from concourse import tile
from concourse.bass_utils import run_bass_kernel_spmd

AF = mybir.ActivationFunctionType
ALU = mybir.AluOpType
DR = mybir.MatmulPerfMode.DoubleRow
f32 = mybir.dt.float32
f32r = mybir.dt.float32r
bf16 = mybir.dt.bfloat16
fp8 = mybir.dt.float8e4

P = 128
DIM = 768
CT = DIM // P            # 6 channel tiles
KP = CT // 2             # 3 channel-tile pairs (DoubleRow)
N = 1024                 # tokens
NT = N // P              # 8 token tiles
NP = NT // 2             # 4 token-tile pairs
NH = 12                  # heads
DH = 64                  # head dim
VW = 80                  # padded head slot width in vsb (stride % 16 == 0)
HID = 3072
HT = HID // P            # 24 hidden tiles
HP = HT // 2             # 12 hidden-tile pairs
B = 8
EPS = 1e-5
SCALE = DH ** -0.5
SCH_A = 8.0 * SCALE * 1.4426950408889634   # fp8e4m3 bits per unit raw-score
SCH_B = 8.0 * 7 - 0.1                      # exponent bias, rounding-robust


def _t6(dram_2d):
    """View a [6*128, M] DRAM tensor/AP as [128, 6, M] (partition-major tiles)."""
    return dram_2d.rearrange("(a p) m -> p a m", p=P)


def build_nc(reps=1):
    nc = bacc.Bacc("TRN2", target_bir_lowering=False, debug=False)

    # ---- I/O ----
    xT = nc.dram_tensor("xT", [DIM, N], f32r, kind="ExternalInput")
    wqk = nc.dram_tensor("wqk", [12, P, CT * P], fp8, kind="ExternalInput")
    wv = nc.dram_tensor("wv", [P, CT, DIM], fp8, kind="ExternalInput")
    wproj = nc.dram_tensor("wproj", [P, CT, DIM], fp8, kind="ExternalInput")
    wfc1 = nc.dram_tensor("wfc1", [HT, P, CT * P], fp8, kind="ExternalInput")
    wfc2 = nc.dram_tensor("wfc2", [HT, P, 2 * 3 * P], fp8, kind="ExternalInput")
    bqk = nc.dram_tensor("bqk", [P, 12], f32, kind="ExternalInput")
    bv = nc.dram_tensor("bv", [DIM], f32, kind="ExternalInput")
    bproj = nc.dram_tensor("bproj", [P, CT], f32, kind="ExternalInput")
    bprojT = nc.dram_tensor("bprojT", [1, DIM], bf16, kind="ExternalInput")
    bfc2T = nc.dram_tensor("bfc2T", [1, DIM], bf16, kind="ExternalInput")
    bfc1 = nc.dram_tensor("bfc1", [P, HT], f32, kind="ExternalInput")
    bfc2 = nc.dram_tensor("bfc2", [P, CT], f32, kind="ExternalInput")
    g1 = nc.dram_tensor("g1", [P, CT], f32, kind="ExternalInput")
    b1 = nc.dram_tensor("b1", [P, CT], f32, kind="ExternalInput")
    g2 = nc.dram_tensor("g2", [P, CT], f32, kind="ExternalInput")
    b2 = nc.dram_tensor("b2", [P, CT], f32, kind="ExternalInput")
    outT = nc.dram_tensor("outT", [DIM, N], f32r, kind="ExternalOutput")

    args = locals()
    with tile.TileContext(nc) as tc:
        _body(nc, tc, args, reps)
    nc.compile()
    return nc


def _body(nc, tc, t, reps=1):
    xT, outT = t["xT"], t["outT"]
    wqk, wv, wproj, wfc1, wfc2 = t["wqk"], t["wv"], t["wproj"], t["wfc1"], t["wfc2"]

    with (
        tc.tile_pool(name="const", bufs=1) as const,
        tc.tile_pool(name="resid", bufs=1) as resid,
        tc.tile_pool(name="hpool", bufs=1) as hpool,
        tc.tile_pool(name="wpool", bufs=1) as wpool,
        tc.tile_pool(name="dram", bufs=1, space="DRAM") as dram,
    ):
        # ---- residual stream (channel-major, fp32) ----
        xsb = resid.tile([P, CT, N], f32r)
        for ct in range(CT):
            nc.sync.dma_start(xsb[:, ct, :], xT[ct * P:(ct + 1) * P, :])

        # ---- constants ----
        ones_ln = const.tile([P, P], f32)
        nc.vector.memset(ones_ln[:], 1.0 / DIM)
        ones_r = const.tile([P, P], f32r)
        nc.scalar.copy(ones_r[:], ones_ln[:])
        ones_b = const.tile([P, P], bf16)
        nc.vector.memset(ones_b[:], 1.0 / DIM)
        eps_t = const.tile([P, 1], f32)
        nc.vector.memset(eps_t[:], EPS)
        bqk_sb = const.tile([P, 12], f32)
        nc.sync.dma_start(bqk_sb[:], t["bqk"][:])
        bproj_sb = const.tile([P, CT], f32)
        nc.sync.dma_start(bproj_sb[:], t["bproj"][:])
        bfc1_sb = const.tile([P, HT], f32)
        nc.sync.dma_start(bfc1_sb[:], t["bfc1"][:])
        bfc2_sb = const.tile([P, CT], f32)
        nc.sync.dma_start(bfc2_sb[:], t["bfc2"][:])
        g1_sb = const.tile([P, CT], f32)
        nc.sync.dma_start(g1_sb[:], t["g1"][:])
        b1_sb = const.tile([P, CT], f32)
        nc.sync.dma_start(b1_sb[:], t["b1"][:])
        g2_sb = const.tile([P, CT], f32)
        nc.sync.dma_start(g2_sb[:], t["g2"][:])
        b2_sb = const.tile([P, CT], f32)
        nc.sync.dma_start(b2_sb[:], t["b2"][:])
        ones_row = const.tile([1, N], bf16)
        nc.vector.memset(ones_row[:], 1.0)
        bprojT_sb = const.tile([1, DIM], bf16)
        nc.sync.dma_start(bprojT_sb[:], t["bprojT"][:])
        bfc2T_sb = const.tile([1, DIM], bf16)
        nc.sync.dma_start(bfc2T_sb[:], t["bfc2T"][:])
        # v-bias broadcast to all partitions
        vb_sb = const.tile([P, DIM], f32)
        bv_ap = t["bv"][:]
        bv_bcast = bass.AP(tensor=bv_ap.tensor, offset=bv_ap.offset,
                           ap=[[0, P], [1, DIM]])
        nc.gpsimd.dma_start(vb_sb[:], bv_bcast)

        # ---- persistent weight buffers (reloaded each rep) ----
        wqk_sb = wpool.tile([P, 12, CT * P], fp8, name="wqk_sb")
        wv_sb = wpool.tile([P, CT, DIM], fp8, name="wv_sb")
        wp_sb = wpool.tile([P, CT, DIM], fp8, name="wp_sb")
        w1_sb = wpool.tile([P, HT, CT * P], fp8, name="w1_sb")
        w2_sb = wpool.tile([P, HT, 2 * 3 * P], fp8, name="w2_sb")
        w2v = w2_sb[:].rearrange("p a (g m) -> p a g m", g=2)

        def layer_norm_T(src, dst, g_sb, b_sb):
            """src: [P, CT, N] fp32; dst: [P, CT, N] fp8 = LN(src) * g + b."""
            with (
                tc.tile_pool(name="ln_tmp", bufs=1) as tmp,
                tc.tile_pool(name="ln_ps", bufs=1, space="PSUM") as lps,
            ):
                mu_ps = lps.tile([P, N], f32)
                e2_ps = lps.tile([P, N], f32)
                for ct in range(CT):
                    sq = tmp.tile([P, N], bf16, tag="sq", bufs=2)
                    nc.gpsimd.tensor_mul(sq[:], src[:, ct, :], src[:, ct, :])
                    for h in range(2):
                        sl = bass.ts(h, 512)
                        nc.tensor.matmul(
                            mu_ps[:, sl], ones_r[:], src[:, ct, sl],
                            start=(ct == 0), stop=(ct == CT - 1))
                        nc.tensor.matmul(
                            e2_ps[:, sl], ones_b[:], sq[:, sl],
                            start=(ct == 0), stop=(ct == CT - 1))
                mu_sb = tmp.tile([P, N], bf16)
                nc.vector.tensor_copy(mu_sb[:], mu_ps[:])
                var = tmp.tile([P, N], bf16)
                nc.vector.tensor_mul(var[:], mu_sb[:], mu_sb[:])
                nc.vector.tensor_sub(var[:], e2_ps[:], var[:])
                rstd = tmp.tile([P, N], bf16)
                nc.scalar.activation(rstd[:], var[:], AF.Sqrt, bias=eps_t[:],
                                     scale=1.0)
                with nc.allow_low_precision(reason="ln rstd bf16"):
                    nc.vector.reciprocal(rstd[:], rstd[:])
                for ct in range(CT):
                    eng = nc.vector if ct % 2 == 0 else nc.gpsimd
                    t1 = tmp.tile([P, N], bf16, tag="t1", bufs=3)
                    eng.tensor_sub(t1[:], src[:, ct, :], mu_sb[:])
                    eng.tensor_mul(t1[:], t1[:], rstd[:])
                    nc.vector.tensor_scalar(
                        out=dst[:, ct, :], in0=t1[:],
                        scalar1=g_sb[:, ct:ct + 1], scalar2=b_sb[:, ct:ct + 1],
                        op0=ALU.mult, op1=ALU.add)

        for _rep in range(reps):
            # weight loads for this rep (big HWDGE transfers)
            nc.sync.dma_start(
                wqk_sb[:], wqk[:].rearrange("a p m -> p a m"))
            nc.scalar.dma_start(wv_sb[:], wv[:])
            nc.scalar.dma_start(wp_sb[:], wproj[:])
            for c in range(3):
                eng = (nc.sync, nc.scalar, nc.sync)[c]
                eng.dma_start(
                    w1_sb[:, c * 8:(c + 1) * 8, :],
                    wfc1[c * 8:(c + 1) * 8, :, :].rearrange(
                        "a p m -> p a m"))
                eng2 = (nc.scalar, nc.sync, nc.scalar)[c]
                eng2.dma_start(
                    w2_sb[:, c * 8:(c + 1) * 8, :],
                    wfc2[c * 8:(c + 1) * 8, :, :].rearrange(
                        "a p m -> p a m"))

            # ======== LN1 (own transient PSUM pool) ========
            h1 = hpool.tile([P, CT, N], fp8, tag="h")
            layer_norm_T(xsb, h1, g1_sb, b1_sb)

            with (
                tc.tile_pool(name="attn", bufs=1) as attn,
                tc.tile_pool(name="att_sb", bufs=1) as asb,
            ):
                vsb = attn.tile([P, NH, NT, VW], fp8, name="vsb")
                osb = attn.tile([P, CT, N], fp8, name="osb")
                qk_sb = attn.tile([P, 12, N], bf16, name="qk_sb")
                dscr = dram.tile([NH, N], bf16, tag="dscr")

                # ones column for the softmax denominators
                nc.vector.memset(vsb[:, :, :, DH], 1.0)

                aps_cm = tc.tile_pool(name="att_ps", bufs=1, space="PSUM")
                aps = aps_cm.__enter__()

                # ==== all 12 q/k matrices (DoubleRow fp8) ====
                for m in range(12):
                    wm = wqk_sb[:, m, :].rearrange("p (c q) -> p c q", q=P)
                    qkps = aps.tile([P, N], f32, tag="sc", bufs=2, name="qkps")
                    for kp in range(KP):
                        for h in range(2):
                            sl = bass.ts(h, 512)
                            nc.tensor.matmul(
                                qkps[:, sl],
                                wm[:, 2 * kp:2 * kp + 2, :],
                                h1[:, 2 * kp:2 * kp + 2, sl],
                                start=(kp == 0), stop=(kp == KP - 1),
                                perf_mode=DR)
                    nc.scalar.activation(
                        qk_sb[:, m, :], qkps[:], AF.Identity,
                        bias=bqk_sb[:, m:m + 1], scale=DS)

                # ---- V production (DoubleRow fp8), interleaved with pair 0
                def v_tile(it):
                    vps = aps.tile([P, N], f32, tag="sc", bufs=2, name="vps")
                    for c0, cn in ((0, 512), (512, 256)):
                        for kp in range(KP):
                            nc.tensor.matmul(
                                vps[:, c0:c0 + cn],
                                h1[:, 2 * kp:2 * kp + 2, it * P:(it + 1) * P],
                                wv_sb[:, 2 * kp:2 * kp + 2, c0:c0 + cn],
                                start=(kp == 0), stop=(kp == KP - 1),
                                perf_mode=DR)
                    nc.vector.scalar_tensor_tensor(
                        out=vsb[:, :, it, 0:DH],
                        in0=vps[:, 0:DIM].rearrange("p (h d) -> p h d", d=DH),
                        scalar=DS, op0=ALU.mult,
                        in1=vb_sb[:].rearrange("p (h d) -> p h d", d=DH),
                        op1=ALU.add)

                def attn_jp(tp, jp, av0, av1):
                    """Scores + exp + DoubleRow AV for key-tile pair jp."""
                    eA = asb.tile([P, 2, N], fp8, tag="e", bufs=4, name="eA")
                    eB = asb.tile([P, 2, N], fp8, tag="e", bufs=4, name="eB")
                    for u in range(2):      # two key tiles in the pair
                        jt = 2 * jp + u
                        js = slice(jt * P, (jt + 1) * P)
                        scA = aps.tile([P, N], f32, tag="sc", bufs=2,
                                       name="scA")
                        scB = aps.tile([P, N], f32, tag="sc", bufs=2,
                                       name="scB")
                        for h in range(2):
                            sl = bass.ts(h, 512)
                            nc.tensor.matmul(
                                scA[:, sl], qk_sb[0:DH, 6 + tp, js],
                                qk_sb[0:DH, tp, sl],
                                tile_position=(0, 0))
                            nc.tensor.matmul(
                                scB[:, sl], qk_sb[DH:P, 6 + tp, js],
                                qk_sb[DH:P, tp, sl],
                                tile_position=(DH, 0))
                        nc.scalar.activation(eA[:, u, :], scA[:], AF.Exp,
                                             scale=SCALE)
                        if u == 0:
                            with nc.allow_low_precision(reason="schraud exp"):
                                nc.vector.tensor_scalar(
                                    out=eB[:, u, :].bitcast(mybir.dt.int8),
                                    in0=scB[:], scalar1=SCH_A, scalar2=SCH_B,
                                    op0=ALU.mult, op1=ALU.add)
                        else:
                            nc.scalar.activation(eB[:, u, :], scB[:], AF.Exp,
                                                 scale=SCALE)
                    for h in range(2):
                        sl = bass.ts(h, 512)
                        nc.tensor.matmul(
                            av0[:, sl],
                            vsb[:, 2 * tp, 2 * jp:2 * jp + 2, 0:DH + 1],
                            eA[:, :, sl],
                            start=(jp == 0), stop=(jp == NP - 1),
                            perf_mode=DR)
                        nc.tensor.matmul(
                            av1[:, sl],
                            vsb[:, 2 * tp + 1, 2 * jp:2 * jp + 2, 0:DH + 1],
                            eB[:, :, sl],
                            start=(jp == 0), stop=(jp == NP - 1),
                            perf_mode=DR)

                def finish_pair(tp, av0, av1):
                    # evict unnormalized o^T and denominators -> DRAM
                    nc.vector.tensor_copy(osb[0:DH, tp, :], av0[0:DH, :])
                    te = asb.tile([DH + 1, N], bf16, tag="tmpo", bufs=2,
                                  name="te")
                    nc.vector.tensor_copy(te[DH:DH + 1, :], av0[DH:DH + 1, :])
                    nc.sync.dma_start(dscr[2 * tp, :], te[DH:DH + 1, :])
                    to = asb.tile([DH + 1, N], fp8, tag="tmpo8", bufs=2,
                                  name="to")
                    tod = asb.tile([DH + 1, N], bf16, tag="tmpo", bufs=2,
                                   name="tod")
                    nc.vector.tensor_copy(to[0:DH, :], av1[0:DH, :])
                    nc.vector.tensor_copy(tod[DH:DH + 1, :], av1[DH:DH + 1, :])
                    nc.sync.dma_start(osb[DH:P, tp, :], to[0:DH, :])
                    nc.sync.dma_start(dscr[2 * tp + 1, :], tod[DH:DH + 1, :])
                    # normalize: Rt = 1/denoms broadcast across partitions
                    Rt = asb.tile([P, N], bf16, tag="Rt", bufs=2, name="Rt")
                    for hh in range(2):
                        srcb = bass.AP(
                            tensor=dscr.tensor,
                            offset=dscr.offset + (2 * tp + hh) * N,
                            ap=[[0, DH], [1, N]])
                        nc.gpsimd.dma_start(Rt[hh * DH:(hh + 1) * DH, :], srcb)
                    with nc.allow_low_precision(reason="softmax denom"):
                        nc.vector.reciprocal(Rt[:], Rt[:])
                    nc.gpsimd.tensor_mul(osb[:, tp, :], osb[:, tp, :], Rt[:])

                # pair 0 interleaved with V production
                av0 = aps.tile([DH + 1, N], f32, tag="av", bufs=2, name="av0")
                av1 = aps.tile([DH + 1, N], f32, tag="av", bufs=2, name="av1")
                for jp in range(NP):
                    v_tile(2 * jp)
                    v_tile(2 * jp + 1)
                    attn_jp(0, jp, av0, av1)
                finish_pair(0, av0, av1)
                for tp in range(1, CT):
                    av0 = aps.tile([DH + 1, N], f32, tag="av", bufs=2,
                                   name="av0")
                    av1 = aps.tile([DH + 1, N], f32, tag="av", bufs=2,
                                   name="av1")
                    for jp in range(NP):
                        attn_jp(tp, jp, av0, av1)
                    finish_pair(tp, av0, av1)
                aps_cm.__exit__(None, None, None)

                # ======== proj (DoubleRow fp8) + residual ========
                with (
                    tc.tile_pool(name="pj_ps", bufs=1, space="PSUM") as pps,
                ):
                    for mt in range(CT):
                        for h in range(2):
                            sl = bass.ts(h, 512)
                            ps = pps.tile([P, 512], f32, tag="ps", bufs=6,
                                          name="ps")
                            nc.tensor.matmul(
                                ps[:], bprojT_sb[:, mt * P:(mt + 1) * P],
                                ones_row[:, sl], start=True, stop=False)
                            for kp in range(KP):
                                nc.tensor.matmul(
                                    ps[:],
                                    wp_sb[:, 2 * kp:2 * kp + 2,
                                          mt * P:(mt + 1) * P],
                                    osb[:, 2 * kp:2 * kp + 2, sl],
                                    start=False, stop=(kp == KP - 1),
                                    perf_mode=DR)
                            nc.vector.scalar_tensor_tensor(
                                out=xsb[:, mt, sl], in0=ps[:],
                                scalar=1.0 / WS, op0=ALU.mult,
                                in1=xsb[:, mt, sl], op1=ALU.add)

            # ======== LN2 + MLP (DoubleRow fp8) ========
            with tc.tile_pool(name="mlp_sb", bufs=1) as mw:
                h2 = hpool.tile([P, CT, N], fp8, tag="h")
                layer_norm_T(xsb, h2, g2_sb, b2_sb)
                h3sb = mw.tile([P, HT, N], fp8, name="h3sb")
                with tc.tile_pool(name="fc1_ps", bufs=1, space="PSUM") as f1p:
                    for ct in range(HT):
                        w1m = w1_sb[:, ct, :].rearrange("p (c q) -> p c q", q=P)
                        h3ps = f1p.tile([P, N], f32, tag="h3ps", bufs=3,
                                        name="h3ps")
                        for kp in range(KP):
                            for h in range(2):
                                sl = bass.ts(h, 512)
                                nc.tensor.matmul(
                                    h3ps[:, sl],
                                    w1m[:, 2 * kp:2 * kp + 2, :],
                                    h2[:, 2 * kp:2 * kp + 2, sl],
                                    start=(kp == 0), stop=(kp == KP - 1),
                                    perf_mode=DR)
                        nc.scalar.activation(
                            h3sb[:, ct, :], h3ps[:], AF.Gelu,
                            bias=bfc1_sb[:, ct:ct + 1], scale=DS)
                with tc.tile_pool(name="fc2_ps", bufs=1, space="PSUM") as f2p:
                    for g in range(2):  # output groups: mt 0-2, 3-5
                        f2ps = [f2p.tile([P, N], f32, tag=f"f2_{i}", bufs=1,
                                         name=f"f2ps{i}") for i in range(3)]
                        for i in range(3):
                            for h in range(2):
                                sl = bass.ts(h, 512)
                                nc.tensor.matmul(
                                    f2ps[i][:, sl],
                                    bfc2T_sb[:, (g * 3 + i) * P:
                                             (g * 3 + i + 1) * P],
                                    ones_row[:, sl], start=True, stop=False)
                        for kp in range(HP):
                            for i in range(3):
                                for h in range(2):
                                    sl = bass.ts(h, 512)
                                    nc.tensor.matmul(
                                        f2ps[i][:, sl],
                                        w2v[:, 2 * kp:2 * kp + 2, g,
                                            i * P:(i + 1) * P],
                                        h3sb[:, 2 * kp:2 * kp + 2, sl],
                                        start=False, stop=(kp == HP - 1),
                                        perf_mode=DR)
                        for i in range(3):
                            mt = g * 3 + i
                            nc.vector.scalar_tensor_tensor(
                                out=xsb[:, mt, :], in0=f2ps[i][:],
                                scalar=1.0 / WS, op0=ALU.mult,
                                in1=xsb[:, mt, :], op1=ALU.add)
                            if _rep == reps - 1:
                                nc.sync.dma_start(
                                    _t6(outT)[:, mt, :], xsb[:, mt, :])


_NC_CACHE = None


def _get_nc():
    global _NC_CACHE
    if _NC_CACHE is None:
        _NC_CACHE = build_nc()
    return _NC_CACHE


WS = 64.0     # weight scale into fp8
HS = 16.0     # LN-output (h) scale into fp8
DS = 1.0 / (WS * HS)


def _f8(a, scale=WS):
    return np.clip(np.asarray(a, np.float32) * scale, -240.0, 240.0).astype(
        ml_dtypes.float8_e4m3)


def _prep_shared(qkv_w, qkv_b, proj_w, proj_b, fc1_w, fc1_b, fc2_w, fc2_b,
                 ln1_g, ln1_b, ln2_g, ln2_b):
    c = lambda a: np.ascontiguousarray(np.asarray(a, dtype=np.float32))
    return {
        "wqk": _f8(np.ascontiguousarray(np.asarray(qkv_w, np.float32)[:, :2 * DIM].reshape(CT, P, 12, P).transpose(2, 1, 0, 3).reshape(12, P, CT * P))),
        "wv": _f8(np.ascontiguousarray(np.asarray(qkv_w, np.float32)[:, 2 * DIM:].reshape(CT, P, DIM).transpose(1, 0, 2))),
        "wproj": _f8(np.ascontiguousarray(np.asarray(proj_w, np.float32).reshape(CT, P, DIM).transpose(1, 0, 2))),
        "wfc1": _f8(np.ascontiguousarray(np.asarray(fc1_w, np.float32).reshape(CT, P, HT, P).transpose(2, 1, 0, 3).reshape(HT, P, CT * P))),
        "wfc2": _f8(np.ascontiguousarray(np.asarray(fc2_w, np.float32).reshape(HT, P, 2 * 3 * P))),
        "bqk": c(np.asarray(qkv_b)[:2 * DIM].reshape(12, P).T),
        "bprojT": np.asarray(proj_b, np.float32).reshape(1, DIM).astype(ml_dtypes.bfloat16) * np.float32(WS),
        "bfc2T": np.asarray(fc2_b, np.float32).reshape(1, DIM).astype(ml_dtypes.bfloat16) * np.float32(WS),
        "bv": c(np.asarray(qkv_b)[2 * DIM:]),
        "bproj": c(np.asarray(proj_b).reshape(CT, P).T),
        "bfc1": c(np.asarray(fc1_b).reshape(HT, P).T),
        "bfc2": c(np.asarray(fc2_b).reshape(CT, P).T),
        "g1": c(np.asarray(ln1_g).reshape(CT, P).T * HS),
        "b1": c(np.asarray(ln1_b).reshape(CT, P).T * HS),
        "g2": c(np.asarray(ln2_g).reshape(CT, P).T * HS),
        "b2": c(np.asarray(ln2_b).reshape(CT, P).T * HS),
    }


def run(x, shared, **spmd_kwargs):
    nc = _get_nc()
    x = np.asarray(x, dtype=np.float32)
    in_maps = [
        {**shared, "xT": np.ascontiguousarray(x[b].T)} for b in range(B)
    ]
    res = run_bass_kernel_spmd(nc, in_maps, core_ids=list(range(B)), **spmd_kwargs)
    out = np.stack([res.results[b]["outT"].T for b in range(B)])
    return out.astype(np.float32), res


def kernel(x, ln1_g, ln1_b, qkv_w, qkv_b, proj_w, proj_b,
           ln2_g, ln2_b, fc1_w, fc1_b, fc2_w, fc2_b):
    shared = _prep_shared(qkv_w, qkv_b, proj_w, proj_b, fc1_w, fc1_b,
                          fc2_w, fc2_b, ln1_g, ln1_b, ln2_g, ln2_b)
    out, _ = run(x, shared)
    return out


# revision 4
# speedup vs baseline: 2.6531x; 2.6531x over previous
"""Trainium2 Bass kernel for a dense pre-norm transformer block (fp8 version).

Problem: x[8, 1024, 768]; per-batch-element transformer block
  (LN1 -> qkv -> 12-head attention -> proj residual -> LN2 -> MLP(gelu) residual).

Strategy (v2):
  - Pure data-parallel: 8 NeuronCores, one batch element each. No collectives.
  - Channel-major activations ([C, tokens]) on device; host transposes.
  - All big GEMMs (q/k/v production, attn@V, proj, fc1, fc2) run fp8-E4M3
    with perf_mode=DoubleRow: 2 contraction tiles per matmul, ~2x PE rate and
    half the weight DMA traffic.  Scores (K=64) stay bf16, packed two heads
    per PE pass via row groups.  PSUM accumulation is fp32 throughout; the
    residual stream and LN statistics stay fp32.
  - LayerNorm stats via ones-matmuls on bitcast f32r (no copies); squares on
    GpSimd; normalize split DVE/GpSimd; final scale-bias on DVE tensor_scalar.
  - All 12 q/k matrices are produced up-front after LN1 so the per-pair
    exp (ScalarE) pipeline runs without PE-side psum contention.
  - exp consumes score PSUM directly and writes fp8 tiles shaped [P, 2, N]
    (two key tiles) which feed DoubleRow attn@V directly; softmax denominators
    ride in a ones-column appended to V (slot padded to 80 bytes for the DR
    16B-stride rule); per-head normalization is broadcast across partitions
    via a small DRAM round-trip, pipelined per head-pair.
  - Weights stream per rep as a handful of large HWDGE DMAs.
"""

import ml_dtypes
import numpy as np

import concourse.bacc as bacc
import concourse.bass as bass
import concourse.mybir as mybir
from concourse import tile
from concourse.bass_utils import run_bass_kernel_spmd

AF = mybir.ActivationFunctionType
ALU = mybir.AluOpType
DR = mybir.MatmulPerfMode.DoubleRow
f32 = mybir.dt.float32
f32r = mybir.dt.float32r
bf16 = mybir.dt.bfloat16
fp8 = mybir.dt.float8e4

P = 128
DIM = 768
CT = DIM // P            # 6 channel tiles
KP = CT // 2             # 3 channel-tile pairs (DoubleRow)
N = 1024                 # tokens
NT = N // P              # 8 token tiles
NP = NT // 2             # 4 token-tile pairs
NH = 12                  # heads
DH = 64                  # head dim
VW = 80                  # padded head slot width in vsb (stride % 16 == 0)
HID = 3072
HT = HID // P            # 24 hidden tiles
HP = HT // 2             # 12 hidden-tile pairs
B = 8
EPS = 1e-5
SCALE = DH ** -0.5
SCH_A = 8.0 * SCALE * 1.4426950408889634   # fp8e4m3 bits per unit raw-score
SCH_B = 8.0 * 7 - 0.1                      # exponent bias, rounding-robust


def _t6(dram_2d):
    """View a [6*128, M] DRAM tensor/AP as [128, 6, M] (partition-major tiles)."""
    return dram_2d.rearrange("(a p) m -> p a m", p=P)


def build_nc(reps=1):
    nc = bacc.Bacc("TRN2", target_bir_lowering=False, debug=False)

    # ---- I/O ----
    xT = nc.dram_tensor("xT", [DIM, N], f32r, kind="ExternalInput")
    wqk = nc.dram_tensor("wqk", [12, P, CT * P], fp8, kind="ExternalInput")
    wv = nc.dram_tensor("wv", [P, CT, DIM], fp8, kind="ExternalInput")
    wproj = nc.dram_tensor("wproj", [P, CT, DIM], fp8, kind="ExternalInput")
    wfc1 = nc.dram_tensor("wfc1", [HT, P, CT * P], fp8, kind="ExternalInput")
    wfc2 = nc.dram_tensor("wfc2", [HT, P, 2 * 3 * P], fp8, kind="ExternalInput")
    bqk = nc.dram_tensor("bqk", [P, 12], f32, kind="ExternalInput")
    bv = nc.dram_tensor("bv", [DIM], f32, kind="ExternalInput")
    bproj = nc.dram_tensor("bproj", [P, CT], f32, kind="ExternalInput")
    bprojT = nc.dram_tensor("bprojT", [1, DIM], bf16, kind="ExternalInput")
    bfc2T = nc.dram_tensor("bfc2T", [1, DIM], bf16, kind="ExternalInput")
    bfc1 = nc.dram_tensor("bfc1", [P, HT], f32, kind="ExternalInput")
    bfc2 = nc.dram_tensor("bfc2", [P, CT], f32, kind="ExternalInput")
    g1 = nc.dram_tensor("g1", [P, CT], f32, kind="ExternalInput")
    b1 = nc.dram_tensor("b1", [P, CT], f32, kind="ExternalInput")
    g2 = nc.dram_tensor("g2", [P, CT], f32, kind="ExternalInput")
    b2 = nc.dram_tensor("b2", [P, CT], f32, kind="ExternalInput")
    outT = nc.dram_tensor("outT", [DIM, N], f32r, kind="ExternalOutput")

    args = locals()
    with tile.TileContext(nc) as tc:
        _body(nc, tc, args, reps)
    nc.compile()
    return nc


def _body(nc, tc, t, reps=1):
    xT, outT = t["xT"], t["outT"]
    wqk, wv, wproj, wfc1, wfc2 = t["wqk"], t["wv"], t["wproj"], t["wfc1"], t["wfc2"]

    with (
        tc.tile_pool(name="const", bufs=1) as const,
        tc.tile_pool(name="resid", bufs=1) as resid,
        tc.tile_pool(name="hpool", bufs=1) as hpool,
        tc.tile_pool(name="wpool", bufs=1) as wpool,
        tc.tile_pool(name="dram", bufs=1, space="DRAM") as dram,
    ):
        # ---- residual stream (channel-major, fp32) ----
        xsb = resid.tile([P, CT, N], f32r)
        for ct in range(CT):
            nc.sync.dma_start(xsb[:, ct, :], xT[ct * P:(ct + 1) * P, :])

        # ---- constants ----
        ones_ln = const.tile([P, P], f32)
        nc.vector.memset(ones_ln[:], 1.0 / DIM)
        ones_r = const.tile([P, P], f32r)
        nc.scalar.copy(ones_r[:], ones_ln[:])
        ones_b = const.tile([P, P], bf16)
        nc.vector.memset(ones_b[:], 1.0 / DIM)
        eps_t = const.tile([P, 1], f32)
        nc.vector.memset(eps_t[:], EPS)
        bqk_sb = const.tile([P, 12], f32)
        nc.sync.dma_start(bqk_sb[:], t["bqk"][:])
        bproj_sb = const.tile([P, CT], f32)
        nc.sync.dma_start(bproj_sb[:], t["bproj"][:])
        bfc1_sb = const.tile([P, HT], f32)
        nc.sync.dma_start(bfc1_sb[:], t["bfc1"][:])
        bfc2_sb = const.tile([P, CT], f32)
        nc.sync.dma_start(bfc2_sb[:], t["bfc2"][:])
        g1_sb = const.tile([P, CT], f32)
        nc.sync.dma_start(g1_sb[:], t["g1"][:])
        b1_sb = const.tile([P, CT], f32)
        nc.sync.dma_start(b1_sb[:], t["b1"][:])
        g2_sb = const.tile([P, CT], f32)
        nc.sync.dma_start(g2_sb[:], t["g2"][:])
        b2_sb = const.tile([P, CT], f32)
        nc.sync.dma_start(b2_sb[:], t["b2"][:])
        ones_row = const.tile([1, N], bf16)
        nc.vector.memset(ones_row[:], 1.0)
        bprojT_sb = const.tile([1, DIM], bf16)
        nc.sync.dma_start(bprojT_sb[:], t["bprojT"][:])
        bfc2T_sb = const.tile([1, DIM], bf16)
        nc.sync.dma_start(bfc2T_sb[:], t["bfc2T"][:])
        # v-bias broadcast to all partitions
        vb_sb = const.tile([P, DIM], f32)
        bv_ap = t["bv"][:]
        bv_bcast = bass.AP(tensor=bv_ap.tensor, offset=bv_ap.offset,
                           ap=[[0, P], [1, DIM]])
        nc.gpsimd.dma_start(vb_sb[:], bv_bcast)

        # ---- persistent weight buffers (reloaded each rep) ----
        wqk_sb = wpool.tile([P, 12, CT * P], fp8, name="wqk_sb")
        wv_sb = wpool.tile([P, CT, DIM], fp8, name="wv_sb")
        wp_sb = wpool.tile([P, CT, DIM], fp8, name="wp_sb")
        w1_sb = wpool.tile([P, HT, CT * P], fp8, name="w1_sb")
        w2_sb = wpool.tile([P, HT, 2 * 3 * P], fp8, name="w2_sb")
        w2v = w2_sb[:].rearrange("p a (g m) -> p a g m", g=2)

        def layer_norm_T(src, dst, g_sb, b_sb):
            """src: [P, CT, N] fp32; dst: [P, CT, N] fp8 = LN(src) * g + b."""
            with (
                tc.tile_pool(name="ln_tmp", bufs=1) as tmp,
                tc.tile_pool(name="ln_ps", bufs=1, space="PSUM") as lps,
            ):
                mu_ps = lps.tile([P, N], f32)
                e2_ps = lps.tile([P, N], f32)
                sqs = []
                for ct in range(CT):
                    sq = tmp.tile([P, N], bf16, tag="sq", bufs=CT)
                    nc.gpsimd.tensor_mul(sq[:], src[:, ct, :], src[:, ct, :])
                    sqs.append(sq)
                    for h in range(2):
                        sl = bass.ts(h, 512)
                        nc.tensor.matmul(
                            mu_ps[:, sl], ones_r[:], src[:, ct, sl],
                            start=(ct == 0), stop=(ct == CT - 1))
                for ct in range(CT):
                    for h in range(2):
                        sl = bass.ts(h, 512)
                        nc.tensor.matmul(
                            e2_ps[:, sl], ones_b[:], sqs[ct][:, sl],
                            start=(ct == 0), stop=(ct == CT - 1))
                mu_sb = tmp.tile([P, N], bf16)
                nc.vector.tensor_copy(mu_sb[:], mu_ps[:])
                var = tmp.tile([P, N], bf16)
                nc.vector.tensor_mul(var[:], mu_sb[:], mu_sb[:])
                nc.vector.tensor_sub(var[:], e2_ps[:], var[:])
                rstd = tmp.tile([P, N], bf16)
                nc.scalar.activation(rstd[:], var[:], AF.Sqrt, bias=eps_t[:],
                                     scale=1.0)
                with nc.allow_low_precision(reason="ln rstd bf16"):
                    nc.vector.reciprocal(rstd[:], rstd[:])
                for ct in range(CT):
                    eng = nc.vector if ct % 2 == 0 else nc.gpsimd
                    t1 = tmp.tile([P, N], bf16, tag="t1", bufs=3)
                    eng.tensor_sub(t1[:], src[:, ct, :], mu_sb[:])
                    eng.tensor_mul(t1[:], t1[:], rstd[:])
                    nc.vector.tensor_scalar(
                        out=dst[:, ct, :], in0=t1[:],
                        scalar1=g_sb[:, ct:ct + 1], scalar2=b_sb[:, ct:ct + 1],
                        op0=ALU.mult, op1=ALU.add)

        for _rep in range(reps):
            # weight loads for this rep (big HWDGE transfers)
            nc.sync.dma_start(
                wqk_sb[:], wqk[:].rearrange("a p m -> p a m"))
            nc.scalar.dma_start(wv_sb[:], wv[:])
            nc.scalar.dma_start(wp_sb[:], wproj[:])
            for c in range(3):
                eng = (nc.sync, nc.scalar, nc.sync)[c]
                eng.dma_start(
                    w1_sb[:, c * 8:(c + 1) * 8, :],
                    wfc1[c * 8:(c + 1) * 8, :, :].rearrange(
                        "a p m -> p a m"))
                eng2 = (nc.scalar, nc.sync, nc.scalar)[c]
                eng2.dma_start(
                    w2_sb[:, c * 8:(c + 1) * 8, :],
                    wfc2[c * 8:(c + 1) * 8, :, :].rearrange(
                        "a p m -> p a m"))

            # ======== LN1 (own transient PSUM pool) ========
            h1 = hpool.tile([P, CT, N], fp8, tag="h")
            layer_norm_T(xsb, h1, g1_sb, b1_sb)

            with (
                tc.tile_pool(name="attn", bufs=1) as attn,
                tc.tile_pool(name="att_sb", bufs=1) as asb,
            ):
                vsb = attn.tile([P, NH, NT, VW], fp8, name="vsb")
                osb = attn.tile([P, CT, N], fp8, name="osb")
                qk_sb = attn.tile([P, 12, N], bf16, name="qk_sb")
                dscr = dram.tile([NH, N], bf16, tag="dscr")

                # ones column for the softmax denominators
                nc.vector.memset(vsb[:, :, :, DH], 1.0)

                aps_cm = tc.tile_pool(name="att_ps", bufs=1, space="PSUM")
                aps = aps_cm.__enter__()

                # ==== all 12 q/k matrices (DoubleRow fp8) ====
                for m in range(12):
                    wm = wqk_sb[:, m, :].rearrange("p (c q) -> p c q", q=P)
                    qkps = aps.tile([P, N], f32, tag="sc", bufs=2, name="qkps")
                    for kp in range(KP):
                        for h in range(2):
                            sl = bass.ts(h, 512)
                            nc.tensor.matmul(
                                qkps[:, sl],
                                wm[:, 2 * kp:2 * kp + 2, :],
                                h1[:, 2 * kp:2 * kp + 2, sl],
                                start=(kp == 0), stop=(kp == KP - 1),
                                perf_mode=DR)
                    nc.scalar.activation(
                        qk_sb[:, m, :], qkps[:], AF.Identity,
                        bias=bqk_sb[:, m:m + 1], scale=DS)

                # ---- V production (DoubleRow fp8), interleaved with pair 0
                def v_tile(it):
                    vps = aps.tile([P, N], f32, tag="sc", bufs=2, name="vps")
                    for kp in range(KP):
                        for c0, cn in ((0, 512), (512, 256)):
                            nc.tensor.matmul(
                                vps[:, c0:c0 + cn],
                                h1[:, 2 * kp:2 * kp + 2, it * P:(it + 1) * P],
                                wv_sb[:, 2 * kp:2 * kp + 2, c0:c0 + cn],
                                start=(kp == 0), stop=(kp == KP - 1),
                                perf_mode=DR)
                    nc.vector.scalar_tensor_tensor(
                        out=vsb[:, :, it, 0:DH],
                        in0=vps[:, 0:DIM].rearrange("p (h d) -> p h d", d=DH),
                        scalar=DS, op0=ALU.mult,
                        in1=vb_sb[:].rearrange("p (h d) -> p h d", d=DH),
                        op1=ALU.add)

                def attn_jp(tp, jp, av0, av1):
                    """Scores + exp + DoubleRow AV for key-tile pair jp."""
                    eA = asb.tile([P, 2, N], fp8, tag="e", bufs=4, name="eA")
                    eB = asb.tile([P, 2, N], fp8, tag="e", bufs=4, name="eB")
                    for u in range(2):      # two key tiles in the pair
                        jt = 2 * jp + u
                        js = slice(jt * P, (jt + 1) * P)
                        scA = aps.tile([P, N], f32, tag="sc", bufs=2,
                                       name="scA")
                        scB = aps.tile([P, N], f32, tag="sc", bufs=2,
                                       name="scB")
                        for h in range(2):
                            sl = bass.ts(h, 512)
                            nc.tensor.matmul(
                                scA[:, sl], qk_sb[0:DH, 6 + tp, js],
                                qk_sb[0:DH, tp, sl],
                                tile_position=(0, 0))
                        for h in range(2):
                            sl = bass.ts(h, 512)
                            nc.tensor.matmul(
                                scB[:, sl], qk_sb[DH:P, 6 + tp, js],
                                qk_sb[DH:P, tp, sl],
                                tile_position=(DH, 0))
                        nc.scalar.activation(eA[:, u, :], scA[:], AF.Exp,
                                             scale=SCALE)
                        if u == 0:
                            with nc.allow_low_precision(reason="schraud exp"):
                                nc.vector.tensor_scalar(
                                    out=eB[:, u, :].bitcast(mybir.dt.int8),
                                    in0=scB[:], scalar1=SCH_A, scalar2=SCH_B,
                                    op0=ALU.mult, op1=ALU.add)
                        else:
                            nc.scalar.activation(eB[:, u, :], scB[:], AF.Exp,
                                                 scale=SCALE)
                    for h in range(2):
                        sl = bass.ts(h, 512)
                        nc.tensor.matmul(
                            av0[:, sl],
                            vsb[:, 2 * tp, 2 * jp:2 * jp + 2, 0:DH + 1],
                            eA[:, :, sl],
                            start=(jp == 0), stop=(jp == NP - 1),
                            perf_mode=DR)
                    for h in range(2):
                        sl = bass.ts(h, 512)
                        nc.tensor.matmul(
                            av1[:, sl],
                            vsb[:, 2 * tp + 1, 2 * jp:2 * jp + 2, 0:DH + 1],
                            eB[:, :, sl],
                            start=(jp == 0), stop=(jp == NP - 1),
                            perf_mode=DR)

                def finish_pair(tp, av0, av1):
                    # evict unnormalized o^T and denominators -> DRAM
                    nc.vector.tensor_copy(osb[0:DH, tp, :], av0[0:DH, :])
                    te = asb.tile([DH + 1, N], bf16, tag="tmpo", bufs=2,
                                  name="te")
                    nc.vector.tensor_copy(te[DH:DH + 1, :], av0[DH:DH + 1, :])
                    nc.sync.dma_start(dscr[2 * tp, :], te[DH:DH + 1, :])
                    to = asb.tile([DH + 1, N], fp8, tag="tmpo8", bufs=2,
                                  name="to")
                    tod = asb.tile([DH + 1, N], bf16, tag="tmpo", bufs=2,
                                   name="tod")
                    nc.vector.tensor_copy(to[0:DH, :], av1[0:DH, :])
                    nc.vector.tensor_copy(tod[DH:DH + 1, :], av1[DH:DH + 1, :])
                    nc.sync.dma_start(osb[DH:P, tp, :], to[0:DH, :])
                    nc.sync.dma_start(dscr[2 * tp + 1, :], tod[DH:DH + 1, :])
                    # normalize: Rt = 1/denoms broadcast across partitions
                    Rt = asb.tile([P, N], bf16, tag="Rt", bufs=2, name="Rt")
                    for hh in range(2):
                        srcb = bass.AP(
                            tensor=dscr.tensor,
                            offset=dscr.offset + (2 * tp + hh) * N,
                            ap=[[0, DH], [1, N]])
                        nc.gpsimd.dma_start(Rt[hh * DH:(hh + 1) * DH, :], srcb)
                    with nc.allow_low_precision(reason="softmax denom"):
                        nc.vector.reciprocal(Rt[:], Rt[:])
                    nc.gpsimd.tensor_mul(osb[:, tp, :], osb[:, tp, :], Rt[:])

                for it in range(NT):
                    v_tile(it)
                av0 = aps.tile([DH + 1, N], f32, tag="av", bufs=2, name="av0")
                av1 = aps.tile([DH + 1, N], f32, tag="av", bufs=2, name="av1")
                for jp in range(NP):
                    attn_jp(0, jp, av0, av1)
                finish_pair(0, av0, av1)
                for tp in range(1, CT):
                    av0 = aps.tile([DH + 1, N], f32, tag="av", bufs=2,
                                   name="av0")
                    av1 = aps.tile([DH + 1, N], f32, tag="av", bufs=2,
                                   name="av1")
                    for jp in range(NP):
                        attn_jp(tp, jp, av0, av1)
                    finish_pair(tp, av0, av1)
                aps_cm.__exit__(None, None, None)

                # ======== proj (DoubleRow fp8) + residual ========
                with (
                    tc.tile_pool(name="pj_ps", bufs=1, space="PSUM") as pps,
                ):
                    for mt in range(CT):
                        pss = [pps.tile([P, 512], f32, tag="ps", bufs=6,
                                        name="ps") for _ in range(2)]
                        for h in range(2):
                            nc.tensor.matmul(
                                pss[h][:], bprojT_sb[:, mt * P:(mt + 1) * P],
                                ones_row[:, bass.ts(h, 512)],
                                start=True, stop=False)
                        for kp in range(KP):
                            for h in range(2):
                                sl = bass.ts(h, 512)
                                nc.tensor.matmul(
                                    pss[h][:],
                                    wp_sb[:, 2 * kp:2 * kp + 2,
                                          mt * P:(mt + 1) * P],
                                    osb[:, 2 * kp:2 * kp + 2, sl],
                                    start=False, stop=(kp == KP - 1),
                                    perf_mode=DR)
                        for h in range(2):
                            sl = bass.ts(h, 512)
                            nc.vector.scalar_tensor_tensor(
                                out=xsb[:, mt, sl], in0=pss[h][:],
                                scalar=1.0 / WS, op0=ALU.mult,
                                in1=xsb[:, mt, sl], op1=ALU.add)

            # ======== LN2 + MLP (DoubleRow fp8) ========
            with tc.tile_pool(name="mlp_sb", bufs=1) as mw:
                h2 = hpool.tile([P, CT, N], fp8, tag="h")
                layer_norm_T(xsb, h2, g2_sb, b2_sb)
                h3sb = mw.tile([P, HT, N], fp8, name="h3sb")
                with tc.tile_pool(name="fc1_ps", bufs=1, space="PSUM") as f1p:
                    for ct in range(HT):
                        w1m = w1_sb[:, ct, :].rearrange("p (c q) -> p c q", q=P)
                        h3ps = f1p.tile([P, N], f32, tag="h3ps", bufs=3,
                                        name="h3ps")
                        for kp in range(KP):
                            for h in range(2):
                                sl = bass.ts(h, 512)
                                nc.tensor.matmul(
                                    h3ps[:, sl],
                                    w1m[:, 2 * kp:2 * kp + 2, :],
                                    h2[:, 2 * kp:2 * kp + 2, sl],
                                    start=(kp == 0), stop=(kp == KP - 1),
                                    perf_mode=DR)
                        nc.scalar.activation(
                            h3sb[:, ct, :], h3ps[:], AF.Gelu,
                            bias=bfc1_sb[:, ct:ct + 1], scale=DS)
                with tc.tile_pool(name="fc2_ps", bufs=1, space="PSUM") as f2p:
                    for g in range(2):  # output groups: mt 0-2, 3-5
                        f2ps = [f2p.tile([P, N], f32, tag=f"f2_{i}", bufs=1,
                                         name=f"f2ps{i}") for i in range(3)]
                        for i in range(3):
                            for h in range(2):
                                sl = bass.ts(h, 512)
                                nc.tensor.matmul(
                                    f2ps[i][:, sl],
                                    bfc2T_sb[:, (g * 3 + i) * P:
                                             (g * 3 + i + 1) * P],
                                    ones_row[:, sl], start=True, stop=False)
                        for kp in range(HP):
                            for i in range(3):
                                for h in range(2):
                                    sl = bass.ts(h, 512)
                                    nc.tensor.matmul(
                                        f2ps[i][:, sl],
                                        w2v[:, 2 * kp:2 * kp + 2, g,
                                            i * P:(i + 1) * P],
                                        h3sb[:, 2 * kp:2 * kp + 2, sl],
                                        start=False, stop=(kp == HP - 1),
                                        perf_mode=DR)
                        for i in range(3):
                            mt = g * 3 + i
                            nc.vector.scalar_tensor_tensor(
                                out=xsb[:, mt, :], in0=f2ps[i][:],
                                scalar=1.0 / WS, op0=ALU.mult,
                                in1=xsb[:, mt, :], op1=ALU.add)
                            if _rep == reps - 1:
                                nc.sync.dma_start(
                                    _t6(outT)[:, mt, :], xsb[:, mt, :])


_NC_CACHE = None


def _get_nc():
    global _NC_CACHE
    if _NC_CACHE is None:
        _NC_CACHE = build_nc()
    return _NC_CACHE


WS = 64.0     # weight scale into fp8
HS = 16.0     # LN-output (h) scale into fp8
DS = 1.0 / (WS * HS)


def _f8(a, scale=WS):
    return np.clip(np.asarray(a, np.float32) * scale, -240.0, 240.0).astype(
        ml_dtypes.float8_e4m3)


def _prep_shared(qkv_w, qkv_b, proj_w, proj_b, fc1_w, fc1_b, fc2_w, fc2_b,
                 ln1_g, ln1_b, ln2_g, ln2_b):
    c = lambda a: np.ascontiguousarray(np.asarray(a, dtype=np.float32))
    return {
        "wqk": _f8(np.ascontiguousarray(np.asarray(qkv_w, np.float32)[:, :2 * DIM].reshape(CT, P, 12, P).transpose(2, 1, 0, 3).reshape(12, P, CT * P))),
        "wv": _f8(np.ascontiguousarray(np.asarray(qkv_w, np.float32)[:, 2 * DIM:].reshape(CT, P, DIM).transpose(1, 0, 2))),
        "wproj": _f8(np.ascontiguousarray(np.asarray(proj_w, np.float32).reshape(CT, P, DIM).transpose(1, 0, 2))),
        "wfc1": _f8(np.ascontiguousarray(np.asarray(fc1_w, np.float32).reshape(CT, P, HT, P).transpose(2, 1, 0, 3).reshape(HT, P, CT * P))),
        "wfc2": _f8(np.ascontiguousarray(np.asarray(fc2_w, np.float32).reshape(HT, P, 2 * 3 * P))),
        "bqk": c(np.asarray(qkv_b)[:2 * DIM].reshape(12, P).T),
        "bprojT": np.asarray(proj_b, np.float32).reshape(1, DIM).astype(ml_dtypes.bfloat16) * np.float32(WS),
        "bfc2T": np.asarray(fc2_b, np.float32).reshape(1, DIM).astype(ml_dtypes.bfloat16) * np.float32(WS),
        "bv": c(np.asarray(qkv_b)[2 * DIM:]),
        "bproj": c(np.asarray(proj_b).reshape(CT, P).T),
        "bfc1": c(np.asarray(fc1_b).reshape(HT, P).T),
        "bfc2": c(np.asarray(fc2_b).reshape(CT, P).T),
        "g1": c(np.asarray(ln1_g).reshape(CT, P).T * HS),
        "b1": c(np.asarray(ln1_b).reshape(CT, P).T * HS),
        "g2": c(np.asarray(ln2_g).reshape(CT, P).T * HS),
        "b2": c(np.asarray(ln2_b).reshape(CT, P).T * HS),
    }


def run(x, shared, **spmd_kwargs):
    nc = _get_nc()
    x = np.asarray(x, dtype=np.float32)
    in_maps = [
        {**shared, "xT": np.ascontiguousarray(x[b].T)} for b in range(B)
    ]
    res = run_bass_kernel_spmd(nc, in_maps, core_ids=list(range(B)), **spmd_kwargs)
    out = np.stack([res.results[b]["outT"].T for b in range(B)])
    return out.astype(np.float32), res


def kernel(x, ln1_g, ln1_b, qkv_w, qkv_b, proj_w, proj_b,
           ln2_g, ln2_b, fc1_w, fc1_b, fc2_w, fc2_b):
    shared = _prep_shared(qkv_w, qkv_b, proj_w, proj_b, fc1_w, fc1_b,
                          fc2_w, fc2_b, ln1_g, ln1_b, ln2_g, ln2_b)
    out, _ = run(x, shared)
    return out


# revision 5
# speedup vs baseline: 2.7177x; 1.0243x over previous
"""Trainium2 Bass kernel for a dense pre-norm transformer block (fp8 version).

Problem: x[8, 1024, 768]; per-batch-element transformer block
  (LN1 -> qkv -> 12-head attention -> proj residual -> LN2 -> MLP(gelu) residual).

Strategy (v2):
  - Pure data-parallel: 8 NeuronCores, one batch element each. No collectives.
  - Channel-major activations ([C, tokens]) on device; host transposes.
  - All big GEMMs (q/k/v production, attn@V, proj, fc1, fc2) run fp8-E4M3
    with perf_mode=DoubleRow: 2 contraction tiles per matmul, ~2x PE rate and
    half the weight DMA traffic.  Scores (K=64) stay bf16, packed two heads
    per PE pass via row groups.  PSUM accumulation is fp32 throughout; the
    residual stream and LN statistics stay fp32.
  - LayerNorm stats via ones-matmuls on bitcast f32r (no copies); squares on
    GpSimd; normalize split DVE/GpSimd; final scale-bias on DVE tensor_scalar.
  - All 12 q/k matrices are produced up-front after LN1 so the per-pair
    exp (ScalarE) pipeline runs without PE-side psum contention.
  - exp consumes score PSUM directly and writes fp8 tiles shaped [P, 2, N]
    (two key tiles) which feed DoubleRow attn@V directly; softmax denominators
    ride in a ones-column appended to V (slot padded to 80 bytes for the DR
    16B-stride rule); per-head normalization is broadcast across partitions
    via a small DRAM round-trip, pipelined per head-pair.
  - Weights stream per rep as a handful of large HWDGE DMAs.
"""

import ml_dtypes
import numpy as np

import concourse.bacc as bacc
import concourse.bass as bass
import concourse.mybir as mybir
from concourse import tile
from concourse.bass_utils import run_bass_kernel_spmd

AF = mybir.ActivationFunctionType
ALU = mybir.AluOpType
DR = mybir.MatmulPerfMode.DoubleRow
f32 = mybir.dt.float32
f32r = mybir.dt.float32r
bf16 = mybir.dt.bfloat16
fp8 = mybir.dt.float8e4

P = 128
DIM = 768
CT = DIM // P            # 6 channel tiles
KP = CT // 2             # 3 channel-tile pairs (DoubleRow)
N = 1024                 # tokens
NT = N // P              # 8 token tiles
NP = NT // 2             # 4 token-tile pairs
NH = 12                  # heads
DH = 64                  # head dim
VW = 80                  # padded head slot width in vsb (stride % 16 == 0)
HID = 3072
HT = HID // P            # 24 hidden tiles
HP = HT // 2             # 12 hidden-tile pairs
B = 8
EPS = 1e-5
SCALE = DH ** -0.5
SCH_A = 8.0 * SCALE * 1.4426950408889634   # fp8e4m3 bits per unit raw-score
SCH_B = 8.0 * 7 - 0.1                      # exponent bias, rounding-robust


def _t6(dram_2d):
    """View a [6*128, M] DRAM tensor/AP as [128, 6, M] (partition-major tiles)."""
    return dram_2d.rearrange("(a p) m -> p a m", p=P)


def build_nc(reps=1):
    nc = bacc.Bacc("TRN2", target_bir_lowering=False, debug=False)

    # ---- I/O ----
    xT = nc.dram_tensor("xT", [DIM, N], f32r, kind="ExternalInput")
    wqk = nc.dram_tensor("wqk", [12, P, CT * P], fp8, kind="ExternalInput")
    wv = nc.dram_tensor("wv", [P, CT, DIM], fp8, kind="ExternalInput")
    wproj = nc.dram_tensor("wproj", [P, CT, DIM], fp8, kind="ExternalInput")
    wfc1 = nc.dram_tensor("wfc1", [HT, P, CT * P], fp8, kind="ExternalInput")
    wfc2 = nc.dram_tensor("wfc2", [HT, P, 2 * 3 * P], fp8, kind="ExternalInput")
    bqk = nc.dram_tensor("bqk", [P, 12], f32, kind="ExternalInput")
    bv = nc.dram_tensor("bv", [DIM], f32, kind="ExternalInput")
    bproj = nc.dram_tensor("bproj", [P, CT], f32, kind="ExternalInput")
    bprojT = nc.dram_tensor("bprojT", [1, DIM], bf16, kind="ExternalInput")
    bfc2T = nc.dram_tensor("bfc2T", [1, DIM], bf16, kind="ExternalInput")
    bfc1 = nc.dram_tensor("bfc1", [P, HT], f32, kind="ExternalInput")
    bfc2 = nc.dram_tensor("bfc2", [P, CT], f32, kind="ExternalInput")
    g1 = nc.dram_tensor("g1", [P, CT], f32, kind="ExternalInput")
    b1 = nc.dram_tensor("b1", [P, CT], f32, kind="ExternalInput")
    g2 = nc.dram_tensor("g2", [P, CT], f32, kind="ExternalInput")
    b2 = nc.dram_tensor("b2", [P, CT], f32, kind="ExternalInput")
    outT = nc.dram_tensor("outT", [DIM, N], f32r, kind="ExternalOutput")

    args = locals()
    with tile.TileContext(nc) as tc:
        _body(nc, tc, args, reps)
    nc.compile()
    return nc


def _body(nc, tc, t, reps=1):
    xT, outT = t["xT"], t["outT"]
    wqk, wv, wproj, wfc1, wfc2 = t["wqk"], t["wv"], t["wproj"], t["wfc1"], t["wfc2"]

    with (
        tc.tile_pool(name="const", bufs=1) as const,
        tc.tile_pool(name="resid", bufs=1) as resid,
        tc.tile_pool(name="hpool", bufs=1) as hpool,
        tc.tile_pool(name="wpool", bufs=1) as wpool,
        tc.tile_pool(name="dram", bufs=1, space="DRAM") as dram,
    ):
        # ---- residual stream (channel-major, fp32) ----
        xsb = resid.tile([P, CT, N], f32r)
        for ct in range(CT):
            nc.sync.dma_start(xsb[:, ct, :], xT[ct * P:(ct + 1) * P, :])

        # ---- constants ----
        ones_ln = const.tile([P, P], f32)
        nc.vector.memset(ones_ln[:], 1.0 / DIM)
        ones_r = const.tile([P, P], f32r)
        nc.scalar.copy(ones_r[:], ones_ln[:])
        ones_b = const.tile([P, P], bf16)
        nc.vector.memset(ones_b[:], 1.0 / DIM)
        eps_t = const.tile([P, 1], f32)
        nc.vector.memset(eps_t[:], EPS)
        bqk_sb = const.tile([P, 12], f32)
        nc.sync.dma_start(bqk_sb[:], t["bqk"][:])
        bproj_sb = const.tile([P, CT], f32)
        nc.sync.dma_start(bproj_sb[:], t["bproj"][:])
        bfc1_sb = const.tile([P, HT], f32)
        nc.sync.dma_start(bfc1_sb[:], t["bfc1"][:])
        bfc2_sb = const.tile([P, CT], f32)
        nc.sync.dma_start(bfc2_sb[:], t["bfc2"][:])
        g1_sb = const.tile([P, CT], f32)
        nc.sync.dma_start(g1_sb[:], t["g1"][:])
        b1_sb = const.tile([P, CT], f32)
        nc.sync.dma_start(b1_sb[:], t["b1"][:])
        g2_sb = const.tile([P, CT], f32)
        nc.sync.dma_start(g2_sb[:], t["g2"][:])
        b2_sb = const.tile([P, CT], f32)
        nc.sync.dma_start(b2_sb[:], t["b2"][:])
        ones_row = const.tile([1, N], bf16)
        nc.vector.memset(ones_row[:], 1.0)
        bprojT_sb = const.tile([1, DIM], bf16)
        nc.sync.dma_start(bprojT_sb[:], t["bprojT"][:])
        bfc2T_sb = const.tile([1, DIM], bf16)
        nc.sync.dma_start(bfc2T_sb[:], t["bfc2T"][:])
        # v-bias broadcast to all partitions
        vb_sb = const.tile([P, DIM], f32)
        bv_ap = t["bv"][:]
        bv_bcast = bass.AP(tensor=bv_ap.tensor, offset=bv_ap.offset,
                           ap=[[0, P], [1, DIM]])
        nc.gpsimd.dma_start(vb_sb[:], bv_bcast)

        # ---- persistent weight buffers (reloaded each rep) ----
        wqk_sb = wpool.tile([P, 12, CT * P], fp8, name="wqk_sb")
        wv_sb = wpool.tile([P, CT, DIM], fp8, name="wv_sb")
        wp_sb = wpool.tile([P, CT, DIM], fp8, name="wp_sb")
        w1_sb = wpool.tile([P, HT, CT * P], fp8, name="w1_sb")
        w2_sb = wpool.tile([P, HT, 2 * 3 * P], fp8, name="w2_sb")
        w2v = w2_sb[:].rearrange("p a (g m) -> p a g m", g=2)

        def layer_norm_T(src, dst, g_sb, b_sb):
            """src: [P, CT, N] fp32; dst: [P, CT, N] fp8 = LN(src) * g + b."""
            with (
                tc.tile_pool(name="ln_tmp", bufs=1) as tmp,
                tc.tile_pool(name="ln_ps", bufs=1, space="PSUM") as lps,
            ):
                mu_ps = lps.tile([P, N], f32)
                e2_ps = lps.tile([P, N], f32)
                sqs = []
                for ct in range(CT):
                    sq = tmp.tile([P, N], bf16, tag="sq", bufs=CT)
                    sqe = nc.vector if ct < 4 else nc.gpsimd
                    sqe.tensor_mul(sq[:], src[:, ct, :], src[:, ct, :])
                    sqs.append(sq)
                    for h in range(2):
                        sl = bass.ts(h, 512)
                        nc.tensor.matmul(
                            mu_ps[:, sl], ones_r[:], src[:, ct, sl],
                            start=(ct == 0), stop=(ct == CT - 1))
                for ct in range(CT):
                    for h in range(2):
                        sl = bass.ts(h, 512)
                        nc.tensor.matmul(
                            e2_ps[:, sl], ones_b[:], sqs[ct][:, sl],
                            start=(ct == 0), stop=(ct == CT - 1))
                mu_sb = tmp.tile([P, N], bf16)
                nc.vector.tensor_copy(mu_sb[:], mu_ps[:])
                var = tmp.tile([P, N], bf16)
                nc.vector.tensor_mul(var[:], mu_sb[:], mu_sb[:])
                nc.vector.tensor_sub(var[:], e2_ps[:], var[:])
                rstd = tmp.tile([P, N], bf16)
                nc.scalar.activation(rstd[:], var[:], AF.Sqrt, bias=eps_t[:],
                                     scale=1.0)
                with nc.allow_low_precision(reason="ln rstd bf16"):
                    nc.vector.reciprocal(rstd[:], rstd[:])
                for ct in range(CT):
                    eng = nc.vector if ct % 2 == 0 else nc.gpsimd
                    t1 = tmp.tile([P, N], bf16, tag="t1", bufs=3)
                    eng.tensor_sub(t1[:], src[:, ct, :], mu_sb[:])
                    eng.tensor_mul(t1[:], t1[:], rstd[:])
                    nc.vector.tensor_scalar(
                        out=dst[:, ct, :], in0=t1[:],
                        scalar1=g_sb[:, ct:ct + 1], scalar2=b_sb[:, ct:ct + 1],
                        op0=ALU.mult, op1=ALU.add)

        for _rep in range(reps):
            # weight loads for this rep (big HWDGE transfers)
            nc.sync.dma_start(
                wqk_sb[:], wqk[:].rearrange("a p m -> p a m"))
            nc.scalar.dma_start(wv_sb[:], wv[:])
            nc.scalar.dma_start(wp_sb[:], wproj[:])
            for c in range(3):
                eng = (nc.sync, nc.scalar, nc.sync)[c]
                eng.dma_start(
                    w1_sb[:, c * 8:(c + 1) * 8, :],
                    wfc1[c * 8:(c + 1) * 8, :, :].rearrange(
                        "a p m -> p a m"))
                eng2 = (nc.scalar, nc.sync, nc.scalar)[c]
                eng2.dma_start(
                    w2_sb[:, c * 8:(c + 1) * 8, :],
                    wfc2[c * 8:(c + 1) * 8, :, :].rearrange(
                        "a p m -> p a m"))

            # ======== LN1 (own transient PSUM pool) ========
            h1 = hpool.tile([P, CT, N], fp8, tag="h")
            layer_norm_T(xsb, h1, g1_sb, b1_sb)

            with (
                tc.tile_pool(name="attn", bufs=1) as attn,
                tc.tile_pool(name="att_sb", bufs=1) as asb,
            ):
                vsb = attn.tile([P, NH, NT, VW], fp8, name="vsb")
                osb = attn.tile([P, CT, N], fp8, name="osb")
                qk_sb = attn.tile([P, 12, N], bf16, name="qk_sb")
                dscr = dram.tile([NH, N], bf16, tag="dscr")

                # ones column for the softmax denominators
                nc.vector.memset(vsb[:, :, :, DH], 1.0)

                aps_cm = tc.tile_pool(name="att_ps", bufs=1, space="PSUM")
                aps = aps_cm.__enter__()

                # ==== all 12 q/k matrices (DoubleRow fp8) ====
                for m in range(12):
                    wm = wqk_sb[:, m, :].rearrange("p (c q) -> p c q", q=P)
                    qkps = aps.tile([P, N], f32, tag="sc", bufs=2, name="qkps")
                    for kp in range(KP):
                        for h in range(2):
                            sl = bass.ts(h, 512)
                            nc.tensor.matmul(
                                qkps[:, sl],
                                wm[:, 2 * kp:2 * kp + 2, :],
                                h1[:, 2 * kp:2 * kp + 2, sl],
                                start=(kp == 0), stop=(kp == KP - 1),
                                perf_mode=DR)
                    nc.scalar.activation(
                        qk_sb[:, m, :], qkps[:], AF.Identity,
                        bias=bqk_sb[:, m:m + 1], scale=DS)

                # ---- V production (DoubleRow fp8), interleaved with pair 0
                def v_tile(it):
                    vps = aps.tile([P, N], f32, tag="sc", bufs=2, name="vps")
                    for kp in range(KP):
                        for c0, cn in ((0, 512), (512, 256)):
                            nc.tensor.matmul(
                                vps[:, c0:c0 + cn],
                                h1[:, 2 * kp:2 * kp + 2, it * P:(it + 1) * P],
                                wv_sb[:, 2 * kp:2 * kp + 2, c0:c0 + cn],
                                start=(kp == 0), stop=(kp == KP - 1),
                                perf_mode=DR)
                    nc.vector.scalar_tensor_tensor(
                        out=vsb[:, :, it, 0:DH],
                        in0=vps[:, 0:DIM].rearrange("p (h d) -> p h d", d=DH),
                        scalar=DS, op0=ALU.mult,
                        in1=vb_sb[:].rearrange("p (h d) -> p h d", d=DH),
                        op1=ALU.add)

                def attn_jp(tp, jp, av0, av1):
                    """Scores + exp + DoubleRow AV for key-tile pair jp."""
                    eA = asb.tile([P, 2, N], fp8, tag="e", bufs=4, name="eA")
                    eB = asb.tile([P, 2, N], fp8, tag="e", bufs=4, name="eB")
                    for u in range(2):      # two key tiles in the pair
                        jt = 2 * jp + u
                        js = slice(jt * P, (jt + 1) * P)
                        scA = aps.tile([P, N], f32, tag="sc", bufs=2,
                                       name="scA")
                        scB = aps.tile([P, N], f32, tag="sc", bufs=2,
                                       name="scB")
                        for h in range(2):
                            sl = bass.ts(h, 512)
                            nc.tensor.matmul(
                                scA[:, sl], qk_sb[0:DH, 6 + tp, js],
                                qk_sb[0:DH, tp, sl],
                                tile_position=(0, 0))
                        for h in range(2):
                            sl = bass.ts(h, 512)
                            nc.tensor.matmul(
                                scB[:, sl], qk_sb[DH:P, 6 + tp, js],
                                qk_sb[DH:P, tp, sl],
                                tile_position=(DH, 0))
                        nc.scalar.activation(eA[:, u, :], scA[:], AF.Exp,
                                             scale=SCALE)
                        if u == 0:
                            with nc.allow_low_precision(reason="schraud exp"):
                                nc.vector.tensor_scalar(
                                    out=eB[:, u, :].bitcast(mybir.dt.int8),
                                    in0=scB[:], scalar1=SCH_A, scalar2=SCH_B,
                                    op0=ALU.mult, op1=ALU.add)
                        else:
                            nc.scalar.activation(eB[:, u, :], scB[:], AF.Exp,
                                                 scale=SCALE)
                    for h in range(2):
                        sl = bass.ts(h, 512)
                        nc.tensor.matmul(
                            av0[:, sl],
                            vsb[:, 2 * tp, 2 * jp:2 * jp + 2, 0:DH + 1],
                            eA[:, :, sl],
                            start=(jp == 0), stop=(jp == NP - 1),
                            perf_mode=DR)
                    for h in range(2):
                        sl = bass.ts(h, 512)
                        nc.tensor.matmul(
                            av1[:, sl],
                            vsb[:, 2 * tp + 1, 2 * jp:2 * jp + 2, 0:DH + 1],
                            eB[:, :, sl],
                            start=(jp == 0), stop=(jp == NP - 1),
                            perf_mode=DR)

                def finish_pair(tp, av0, av1):
                    # evict unnormalized o^T and denominators -> DRAM
                    nc.vector.tensor_copy(osb[0:DH, tp, :], av0[0:DH, :])
                    te = asb.tile([DH + 1, N], bf16, tag="tmpo", bufs=2,
                                  name="te")
                    nc.vector.tensor_copy(te[DH:DH + 1, :], av0[DH:DH + 1, :])
                    nc.sync.dma_start(dscr[2 * tp, :], te[DH:DH + 1, :])
                    to = asb.tile([DH + 1, N], fp8, tag="tmpo8", bufs=2,
                                  name="to")
                    tod = asb.tile([DH + 1, N], bf16, tag="tmpo", bufs=2,
                                   name="tod")
                    nc.vector.tensor_copy(to[0:DH, :], av1[0:DH, :])
                    nc.vector.tensor_copy(tod[DH:DH + 1, :], av1[DH:DH + 1, :])
                    nc.sync.dma_start(osb[DH:P, tp, :], to[0:DH, :])
                    nc.sync.dma_start(dscr[2 * tp + 1, :], tod[DH:DH + 1, :])
                    # normalize: Rt = 1/denoms broadcast across partitions
                    Rt = asb.tile([P, N], bf16, tag="Rt", bufs=2, name="Rt")
                    for hh in range(2):
                        srcb = bass.AP(
                            tensor=dscr.tensor,
                            offset=dscr.offset + (2 * tp + hh) * N,
                            ap=[[0, DH], [1, N]])
                        nc.gpsimd.dma_start(Rt[hh * DH:(hh + 1) * DH, :], srcb)
                    with nc.allow_low_precision(reason="softmax denom"):
                        nc.vector.reciprocal(Rt[:], Rt[:])
                    nc.gpsimd.tensor_mul(osb[:, tp, :], osb[:, tp, :], Rt[:])

                for it in range(NT):
                    v_tile(it)
                av0 = aps.tile([DH + 1, N], f32, tag="av", bufs=2, name="av0")
                av1 = aps.tile([DH + 1, N], f32, tag="av", bufs=2, name="av1")
                for jp in range(NP):
                    attn_jp(0, jp, av0, av1)
                finish_pair(0, av0, av1)
                for tp in range(1, CT):
                    av0 = aps.tile([DH + 1, N], f32, tag="av", bufs=2,
                                   name="av0")
                    av1 = aps.tile([DH + 1, N], f32, tag="av", bufs=2,
                                   name="av1")
                    for jp in range(NP):
                        attn_jp(tp, jp, av0, av1)
                    finish_pair(tp, av0, av1)
                aps_cm.__exit__(None, None, None)

                # ======== proj (DoubleRow fp8) + residual ========
                with (
                    tc.tile_pool(name="pj_ps", bufs=1, space="PSUM") as pps,
                ):
                    for mt in range(CT):
                        pss = [pps.tile([P, 512], f32, tag="ps", bufs=6,
                                        name="ps") for _ in range(2)]
                        for h in range(2):
                            nc.tensor.matmul(
                                pss[h][:], bprojT_sb[:, mt * P:(mt + 1) * P],
                                ones_row[:, bass.ts(h, 512)],
                                start=True, stop=False)
                        for kp in range(KP):
                            for h in range(2):
                                sl = bass.ts(h, 512)
                                nc.tensor.matmul(
                                    pss[h][:],
                                    wp_sb[:, 2 * kp:2 * kp + 2,
                                          mt * P:(mt + 1) * P],
                                    osb[:, 2 * kp:2 * kp + 2, sl],
                                    start=False, stop=(kp == KP - 1),
                                    perf_mode=DR)
                        for h in range(2):
                            sl = bass.ts(h, 512)
                            nc.vector.scalar_tensor_tensor(
                                out=xsb[:, mt, sl], in0=pss[h][:],
                                scalar=1.0 / WS, op0=ALU.mult,
                                in1=xsb[:, mt, sl], op1=ALU.add)

            # ======== LN2 + MLP (DoubleRow fp8) ========
            with tc.tile_pool(name="mlp_sb", bufs=1) as mw:
                h2 = hpool.tile([P, CT, N], fp8, tag="h")
                layer_norm_T(xsb, h2, g2_sb, b2_sb)
                h3sb = mw.tile([P, HT, N], fp8, name="h3sb")
                with tc.tile_pool(name="fc1_ps", bufs=1, space="PSUM") as f1p:
                    for ct in range(HT):
                        w1m = w1_sb[:, ct, :].rearrange("p (c q) -> p c q", q=P)
                        h3ps = f1p.tile([P, N], f32, tag="h3ps", bufs=3,
                                        name="h3ps")
                        for kp in range(KP):
                            for h in range(2):
                                sl = bass.ts(h, 512)
                                nc.tensor.matmul(
                                    h3ps[:, sl],
                                    w1m[:, 2 * kp:2 * kp + 2, :],
                                    h2[:, 2 * kp:2 * kp + 2, sl],
                                    start=(kp == 0), stop=(kp == KP - 1),
                                    perf_mode=DR)
                        nc.scalar.activation(
                            h3sb[:, ct, :], h3ps[:], AF.Gelu,
                            bias=bfc1_sb[:, ct:ct + 1], scale=DS)
                with tc.tile_pool(name="fc2_ps", bufs=1, space="PSUM") as f2p:
                    for g in range(2):  # output groups: mt 0-2, 3-5
                        f2ps = [f2p.tile([P, N], f32, tag=f"f2_{i}", bufs=1,
                                         name=f"f2ps{i}") for i in range(3)]
                        for i in range(3):
                            for h in range(2):
                                sl = bass.ts(h, 512)
                                nc.tensor.matmul(
                                    f2ps[i][:, sl],
                                    bfc2T_sb[:, (g * 3 + i) * P:
                                             (g * 3 + i + 1) * P],
                                    ones_row[:, sl], start=True, stop=False)
                        for kp in range(HP):
                            for i in range(3):
                                for h in range(2):
                                    sl = bass.ts(h, 512)
                                    nc.tensor.matmul(
                                        f2ps[i][:, sl],
                                        w2v[:, 2 * kp:2 * kp + 2, g,
                                            i * P:(i + 1) * P],
                                        h3sb[:, 2 * kp:2 * kp + 2, sl],
                                        start=False, stop=(kp == HP - 1),
                                        perf_mode=DR)
                        for i in range(3):
                            mt = g * 3 + i
                            nc.vector.scalar_tensor_tensor(
                                out=xsb[:, mt, :], in0=f2ps[i][:],
                                scalar=1.0 / WS, op0=ALU.mult,
                                in1=xsb[:, mt, :], op1=ALU.add)
                            if _rep == reps - 1:
                                nc.sync.dma_start(
                                    _t6(outT)[:, mt, :], xsb[:, mt, :])


_NC_CACHE = None


def _get_nc():
    global _NC_CACHE
    if _NC_CACHE is None:
        _NC_CACHE = build_nc()
    return _NC_CACHE


WS = 64.0     # weight scale into fp8
HS = 16.0     # LN-output (h) scale into fp8
DS = 1.0 / (WS * HS)


def _f8(a, scale=WS):
    return np.clip(np.asarray(a, np.float32) * scale, -240.0, 240.0).astype(
        ml_dtypes.float8_e4m3)


def _prep_shared(qkv_w, qkv_b, proj_w, proj_b, fc1_w, fc1_b, fc2_w, fc2_b,
                 ln1_g, ln1_b, ln2_g, ln2_b):
    c = lambda a: np.ascontiguousarray(np.asarray(a, dtype=np.float32))
    return {
        "wqk": _f8(np.ascontiguousarray(np.asarray(qkv_w, np.float32)[:, :2 * DIM].reshape(CT, P, 12, P).transpose(2, 1, 0, 3).reshape(12, P, CT * P))),
        "wv": _f8(np.ascontiguousarray(np.asarray(qkv_w, np.float32)[:, 2 * DIM:].reshape(CT, P, DIM).transpose(1, 0, 2))),
        "wproj": _f8(np.ascontiguousarray(np.asarray(proj_w, np.float32).reshape(CT, P, DIM).transpose(1, 0, 2))),
        "wfc1": _f8(np.ascontiguousarray(np.asarray(fc1_w, np.float32).reshape(CT, P, HT, P).transpose(2, 1, 0, 3).reshape(HT, P, CT * P))),
        "wfc2": _f8(np.ascontiguousarray(np.asarray(fc2_w, np.float32).reshape(HT, P, 2 * 3 * P))),
        "bqk": c(np.asarray(qkv_b)[:2 * DIM].reshape(12, P).T),
        "bprojT": np.asarray(proj_b, np.float32).reshape(1, DIM).astype(ml_dtypes.bfloat16) * np.float32(WS),
        "bfc2T": np.asarray(fc2_b, np.float32).reshape(1, DIM).astype(ml_dtypes.bfloat16) * np.float32(WS),
        "bv": c(np.asarray(qkv_b)[2 * DIM:]),
        "bproj": c(np.asarray(proj_b).reshape(CT, P).T),
        "bfc1": c(np.asarray(fc1_b).reshape(HT, P).T),
        "bfc2": c(np.asarray(fc2_b).reshape(CT, P).T),
        "g1": c(np.asarray(ln1_g).reshape(CT, P).T * HS),
        "b1": c(np.asarray(ln1_b).reshape(CT, P).T * HS),
        "g2": c(np.asarray(ln2_g).reshape(CT, P).T * HS),
        "b2": c(np.asarray(ln2_b).reshape(CT, P).T * HS),
    }


def run(x, shared, **spmd_kwargs):
    nc = _get_nc()
    x = np.asarray(x, dtype=np.float32)
    in_maps = [
        {**shared, "xT": np.ascontiguousarray(x[b].T)} for b in range(B)
    ]
    res = run_bass_kernel_spmd(nc, in_maps, core_ids=list(range(B)), **spmd_kwargs)
    out = np.stack([res.results[b]["outT"].T for b in range(B)])
    return out.astype(np.float32), res


def kernel(x, ln1_g, ln1_b, qkv_w, qkv_b, proj_w, proj_b,
           ln2_g, ln2_b, fc1_w, fc1_b, fc2_w, fc2_b):
    shared = _prep_shared(qkv_w, qkv_b, proj_w, proj_b, fc1_w, fc1_b,
                          fc2_w, fc2_b, ln1_g, ln1_b, ln2_g, ln2_b)
    out, _ = run(x, shared)
    return out


# revision 6
# speedup vs baseline: 2.7792x; 1.0226x over previous
"""Trainium2 Bass kernel for a dense pre-norm transformer block (fp8 version).

Problem: x[8, 1024, 768]; per-batch-element transformer block
  (LN1 -> qkv -> 12-head attention -> proj residual -> LN2 -> MLP(gelu) residual).

Strategy (v2):
  - Pure data-parallel: 8 NeuronCores, one batch element each. No collectives.
  - Channel-major activations ([C, tokens]) on device; host transposes.
  - All big GEMMs (q/k/v production, attn@V, proj, fc1, fc2) run fp8-E4M3
    with perf_mode=DoubleRow: 2 contraction tiles per matmul, ~2x PE rate and
    half the weight DMA traffic.  Scores (K=64) stay bf16, packed two heads
    per PE pass via row groups.  PSUM accumulation is fp32 throughout; the
    residual stream and LN statistics stay fp32.
  - LayerNorm stats via ones-matmuls on bitcast f32r (no copies); squares on
    GpSimd; normalize split DVE/GpSimd; final scale-bias on DVE tensor_scalar.
  - All 12 q/k matrices are produced up-front after LN1 so the per-pair
    exp (ScalarE) pipeline runs without PE-side psum contention.
  - exp consumes score PSUM directly and writes fp8 tiles shaped [P, 2, N]
    (two key tiles) which feed DoubleRow attn@V directly; softmax denominators
    ride in a ones-column appended to V (slot padded to 80 bytes for the DR
    16B-stride rule); per-head normalization is broadcast across partitions
    via a small DRAM round-trip, pipelined per head-pair.
  - Weights stream per rep as a handful of large HWDGE DMAs.
"""

import ml_dtypes
import numpy as np

import concourse.bacc as bacc
import concourse.bass as bass
import concourse.mybir as mybir
from concourse import tile
from concourse.bass_utils import run_bass_kernel_spmd

AF = mybir.ActivationFunctionType
ALU = mybir.AluOpType
DR = mybir.MatmulPerfMode.DoubleRow
f32 = mybir.dt.float32
f32r = mybir.dt.float32r
bf16 = mybir.dt.bfloat16
fp8 = mybir.dt.float8e4

P = 128
DIM = 768
CT = DIM // P            # 6 channel tiles
KP = CT // 2             # 3 channel-tile pairs (DoubleRow)
N = 1024                 # tokens
NT = N // P              # 8 token tiles
NP = NT // 2             # 4 token-tile pairs
NH = 12                  # heads
DH = 64                  # head dim
VW = 80                  # padded head slot width in vsb (stride % 16 == 0)
HID = 3072
HT = HID // P            # 24 hidden tiles
HP = HT // 2             # 12 hidden-tile pairs
B = 8
EPS = 1e-5
SCALE = DH ** -0.5
SCH_A = 8.0 * SCALE * 1.4426950408889634   # fp8e4m3 bits per unit raw-score
SCH_B = 8.0 * 7 - 0.1                      # exponent bias, rounding-robust


def _t6(dram_2d):
    """View a [6*128, M] DRAM tensor/AP as [128, 6, M] (partition-major tiles)."""
    return dram_2d.rearrange("(a p) m -> p a m", p=P)


def build_nc(reps=1):
    nc = bacc.Bacc("TRN2", target_bir_lowering=False, debug=False)

    # ---- I/O ----
    xT = nc.dram_tensor("xT", [DIM, N], f32r, kind="ExternalInput")
    wqk = nc.dram_tensor("wqk", [12, P, CT * P], fp8, kind="ExternalInput")
    wv = nc.dram_tensor("wv", [P, CT, DIM], fp8, kind="ExternalInput")
    wproj = nc.dram_tensor("wproj", [P, CT, DIM], fp8, kind="ExternalInput")
    wfc1 = nc.dram_tensor("wfc1", [HT, P, CT * P], fp8, kind="ExternalInput")
    wfc2 = nc.dram_tensor("wfc2", [HT, P, 2 * 3 * P], fp8, kind="ExternalInput")
    bqk = nc.dram_tensor("bqk", [P, 12], f32, kind="ExternalInput")
    bv = nc.dram_tensor("bv", [DIM], f32, kind="ExternalInput")
    bproj = nc.dram_tensor("bproj", [P, CT], f32, kind="ExternalInput")
    bprojT = nc.dram_tensor("bprojT", [1, DIM], bf16, kind="ExternalInput")
    bfc2T = nc.dram_tensor("bfc2T", [1, DIM], bf16, kind="ExternalInput")
    bfc1 = nc.dram_tensor("bfc1", [P, HT], f32, kind="ExternalInput")
    bfc2 = nc.dram_tensor("bfc2", [P, CT], f32, kind="ExternalInput")
    g1 = nc.dram_tensor("g1", [P, CT], f32, kind="ExternalInput")
    b1 = nc.dram_tensor("b1", [P, CT], f32, kind="ExternalInput")
    g2 = nc.dram_tensor("g2", [P, CT], f32, kind="ExternalInput")
    b2 = nc.dram_tensor("b2", [P, CT], f32, kind="ExternalInput")
    outT = nc.dram_tensor("outT", [DIM, N], f32r, kind="ExternalOutput")

    args = locals()
    with tile.TileContext(nc) as tc:
        _body(nc, tc, args, reps)
    nc.compile()
    return nc


def _body(nc, tc, t, reps=1):
    xT, outT = t["xT"], t["outT"]
    wqk, wv, wproj, wfc1, wfc2 = t["wqk"], t["wv"], t["wproj"], t["wfc1"], t["wfc2"]

    with (
        tc.tile_pool(name="const", bufs=1) as const,
        tc.tile_pool(name="resid", bufs=1) as resid,
        tc.tile_pool(name="hpool", bufs=1) as hpool,
        tc.tile_pool(name="wpool", bufs=1) as wpool,
        tc.tile_pool(name="dram", bufs=1, space="DRAM") as dram,
    ):
        # ---- residual stream (channel-major, fp32) ----
        xsb = resid.tile([P, CT, N], f32r)
        for ct in range(CT):
            nc.sync.dma_start(xsb[:, ct, :], xT[ct * P:(ct + 1) * P, :])

        # ---- constants ----
        ones_ln = const.tile([P, P], f32)
        nc.vector.memset(ones_ln[:], 1.0 / DIM)
        ones_r = const.tile([P, P], f32r)
        nc.scalar.copy(ones_r[:], ones_ln[:])
        ones_b = const.tile([P, P], bf16)
        nc.vector.memset(ones_b[:], 1.0 / DIM)
        eps_t = const.tile([P, 1], f32)
        nc.vector.memset(eps_t[:], EPS)
        bqk_sb = const.tile([P, 12], f32)
        nc.sync.dma_start(bqk_sb[:], t["bqk"][:])
        bproj_sb = const.tile([P, CT], f32)
        nc.sync.dma_start(bproj_sb[:], t["bproj"][:])
        bfc1_sb = const.tile([P, HT], f32)
        nc.sync.dma_start(bfc1_sb[:], t["bfc1"][:])
        bfc2_sb = const.tile([P, CT], f32)
        nc.sync.dma_start(bfc2_sb[:], t["bfc2"][:])
        g1_sb = const.tile([P, CT], f32)
        nc.sync.dma_start(g1_sb[:], t["g1"][:])
        b1_sb = const.tile([P, CT], f32)
        nc.sync.dma_start(b1_sb[:], t["b1"][:])
        g2_sb = const.tile([P, CT], f32)
        nc.sync.dma_start(g2_sb[:], t["g2"][:])
        b2_sb = const.tile([P, CT], f32)
        nc.sync.dma_start(b2_sb[:], t["b2"][:])
        ones_row = const.tile([1, N], bf16)
        nc.vector.memset(ones_row[:], 1.0)
        bprojT_sb = const.tile([1, DIM], bf16)
        nc.sync.dma_start(bprojT_sb[:], t["bprojT"][:])
        bfc2T_sb = const.tile([1, DIM], bf16)
        nc.sync.dma_start(bfc2T_sb[:], t["bfc2T"][:])
        # v-bias broadcast to all partitions
        vb_sb = const.tile([P, DIM], f32)
        bv_ap = t["bv"][:]
        bv_bcast = bass.AP(tensor=bv_ap.tensor, offset=bv_ap.offset,
                           ap=[[0, P], [1, DIM]])
        nc.gpsimd.dma_start(vb_sb[:], bv_bcast)

        # ---- persistent weight buffers (reloaded each rep) ----
        wqk_sb = wpool.tile([P, 12, CT * P], fp8, name="wqk_sb")
        wv_sb = wpool.tile([P, CT, DIM], fp8, name="wv_sb")
        wp_sb = wpool.tile([P, CT, DIM], fp8, name="wp_sb")
        w1_sb = wpool.tile([P, HT, CT * P], fp8, name="w1_sb")
        w2_sb = wpool.tile([P, HT, 2 * 3 * P], fp8, name="w2_sb")
        w2v = w2_sb[:].rearrange("p a (g m) -> p a g m", g=2)

        def layer_norm_T(src, dst, g_sb, b_sb):
            """src: [P, CT, N] fp32; dst: [P, CT, N] fp8 = LN(src) * g + b."""
            with (
                tc.tile_pool(name="ln_tmp", bufs=1) as tmp,
                tc.tile_pool(name="ln_ps", bufs=1, space="PSUM") as lps,
            ):
                mu_ps = lps.tile([P, N], f32)
                e2_ps = lps.tile([P, N], f32)
                sqs = []
                for ct in range(CT):
                    sq = tmp.tile([P, N], bf16, tag="sq", bufs=CT)
                    sqe = nc.vector if ct < 4 else nc.gpsimd
                    sqe.tensor_mul(sq[:], src[:, ct, :], src[:, ct, :])
                    sqs.append(sq)
                    for h in range(2):
                        sl = bass.ts(h, 512)
                        nc.tensor.matmul(
                            mu_ps[:, sl], ones_r[:], src[:, ct, sl],
                            start=(ct == 0), stop=(ct == CT - 1))
                for ct in range(CT):
                    for h in range(2):
                        sl = bass.ts(h, 512)
                        nc.tensor.matmul(
                            e2_ps[:, sl], ones_b[:], sqs[ct][:, sl],
                            start=(ct == 0), stop=(ct == CT - 1))
                mu_sb = tmp.tile([P, N], bf16)
                nc.vector.tensor_copy(mu_sb[:], mu_ps[:])
                var = tmp.tile([P, N], bf16)
                nc.vector.tensor_mul(var[:], mu_sb[:], mu_sb[:])
                nc.vector.tensor_sub(var[:], e2_ps[:], var[:])
                rstd = tmp.tile([P, N], bf16)
                nc.scalar.activation(rstd[:], var[:], AF.Sqrt, bias=eps_t[:],
                                     scale=1.0)
                with nc.allow_low_precision(reason="ln rstd bf16"):
                    nc.vector.reciprocal(rstd[:], rstd[:])
                for ct in range(CT):
                    eng = nc.vector if ct % 2 == 0 else nc.gpsimd
                    t1 = tmp.tile([P, N], bf16, tag="t1", bufs=3)
                    eng.tensor_sub(t1[:], src[:, ct, :], mu_sb[:])
                    eng.tensor_mul(t1[:], t1[:], rstd[:])
                    nc.vector.tensor_scalar(
                        out=dst[:, ct, :], in0=t1[:],
                        scalar1=g_sb[:, ct:ct + 1], scalar2=b_sb[:, ct:ct + 1],
                        op0=ALU.mult, op1=ALU.add)

        for _rep in range(reps):
            # weight loads for this rep (big HWDGE transfers)
            nc.sync.dma_start(
                wqk_sb[:], wqk[:].rearrange("a p m -> p a m"))
            nc.scalar.dma_start(wv_sb[:], wv[:])
            nc.scalar.dma_start(wp_sb[:], wproj[:])
            for c in range(3):
                eng = (nc.sync, nc.scalar, nc.sync)[c]
                eng.dma_start(
                    w1_sb[:, c * 8:(c + 1) * 8, :],
                    wfc1[c * 8:(c + 1) * 8, :, :].rearrange(
                        "a p m -> p a m"))
                eng2 = (nc.scalar, nc.sync, nc.scalar)[c]
                eng2.dma_start(
                    w2_sb[:, c * 8:(c + 1) * 8, :],
                    wfc2[c * 8:(c + 1) * 8, :, :].rearrange(
                        "a p m -> p a m"))

            # ======== LN1 (own transient PSUM pool) ========
            h1 = hpool.tile([P, CT, N], fp8, tag="h")
            layer_norm_T(xsb, h1, g1_sb, b1_sb)

            with (
                tc.tile_pool(name="attn", bufs=1) as attn,
                tc.tile_pool(name="att_sb", bufs=1) as asb,
            ):
                vsb = attn.tile([P, NH, NT, VW], fp8, name="vsb")
                osb = attn.tile([P, CT, N], fp8, name="osb")
                qk_sb = attn.tile([P, 12, N], bf16, name="qk_sb")
                dscr = dram.tile([NH, N], bf16, tag="dscr")

                # ones column for the softmax denominators
                nc.vector.memset(vsb[:, :, :, DH], 1.0)

                aps_cm = tc.tile_pool(name="att_ps", bufs=1, space="PSUM")
                aps = aps_cm.__enter__()

                # ==== all 12 q/k matrices (DoubleRow fp8) ====
                for m in range(12):
                    wm = wqk_sb[:, m, :].rearrange("p (c q) -> p c q", q=P)
                    qkps = aps.tile([P, N], f32, tag="sc", bufs=2, name="qkps")
                    for kp in range(KP):
                        for h in range(2):
                            sl = bass.ts(h, 512)
                            nc.tensor.matmul(
                                qkps[:, sl],
                                wm[:, 2 * kp:2 * kp + 2, :],
                                h1[:, 2 * kp:2 * kp + 2, sl],
                                start=(kp == 0), stop=(kp == KP - 1),
                                perf_mode=DR)
                    nc.scalar.activation(
                        qk_sb[:, m, :], qkps[:], AF.Identity,
                        bias=bqk_sb[:, m:m + 1], scale=DS)

                # ---- V production (DoubleRow fp8), interleaved with pair 0
                def v_tile(it):
                    vps = aps.tile([P, N], f32, tag="sc", bufs=2, name="vps")
                    for kp in range(KP):
                        for c0, cn in ((0, 512), (512, 256)):
                            nc.tensor.matmul(
                                vps[:, c0:c0 + cn],
                                h1[:, 2 * kp:2 * kp + 2, it * P:(it + 1) * P],
                                wv_sb[:, 2 * kp:2 * kp + 2, c0:c0 + cn],
                                start=(kp == 0), stop=(kp == KP - 1),
                                perf_mode=DR)
                    nc.vector.scalar_tensor_tensor(
                        out=vsb[:, :, it, 0:DH],
                        in0=vps[:, 0:DIM].rearrange("p (h d) -> p h d", d=DH),
                        scalar=DS, op0=ALU.mult,
                        in1=vb_sb[:].rearrange("p (h d) -> p h d", d=DH),
                        op1=ALU.add)

                def attn_jp(tp, jp, av0, av1):
                    """Scores + exp + DoubleRow AV for key-tile pair jp."""
                    eA = asb.tile([P, 2, N], fp8, tag="e", bufs=4, name="eA")
                    eB = asb.tile([P, 2, N], fp8, tag="e", bufs=4, name="eB")
                    for u in range(2):      # two key tiles in the pair
                        jt = 2 * jp + u
                        js = slice(jt * P, (jt + 1) * P)
                        scA = aps.tile([P, N], f32, tag="sc", bufs=2,
                                       name="scA")
                        scB = aps.tile([P, N], f32, tag="sc", bufs=2,
                                       name="scB")
                        for h in range(2):
                            sl = bass.ts(h, 512)
                            nc.tensor.matmul(
                                scA[:, sl], qk_sb[0:DH, 6 + tp, js],
                                qk_sb[0:DH, tp, sl],
                                tile_position=(0, 0))
                        for h in range(2):
                            sl = bass.ts(h, 512)
                            nc.tensor.matmul(
                                scB[:, sl], qk_sb[DH:P, 6 + tp, js],
                                qk_sb[DH:P, tp, sl],
                                tile_position=(DH, 0))
                        nc.scalar.activation(eA[:, u, :], scA[:], AF.Exp,
                                             scale=SCALE)
                        if u == 0:
                            with nc.allow_low_precision(reason="schraud exp"):
                                nc.vector.tensor_scalar(
                                    out=eB[:, u, :].bitcast(mybir.dt.int8),
                                    in0=scB[:], scalar1=SCH_A, scalar2=SCH_B,
                                    op0=ALU.mult, op1=ALU.add)
                        else:
                            nc.scalar.activation(eB[:, u, :], scB[:], AF.Exp,
                                                 scale=SCALE)
                    for h in range(2):
                        sl = bass.ts(h, 512)
                        nc.tensor.matmul(
                            av0[:, sl],
                            vsb[:, 2 * tp, 2 * jp:2 * jp + 2, 0:DH + 1],
                            eA[:, :, sl],
                            start=(jp == 0), stop=(jp == NP - 1),
                            perf_mode=DR)
                    for h in range(2):
                        sl = bass.ts(h, 512)
                        nc.tensor.matmul(
                            av1[:, sl],
                            vsb[:, 2 * tp + 1, 2 * jp:2 * jp + 2, 0:DH + 1],
                            eB[:, :, sl],
                            start=(jp == 0), stop=(jp == NP - 1),
                            perf_mode=DR)

                def finish_pair(tp, av0, av1):
                    # evict unnormalized o^T + denominators in one bf16 copy
                    # per head; SWDGE cast-DMAs (bf16->fp8) deliver osb.
                    teb = asb.tile([DH + 1, N], bf16, tag="tmpo", bufs=3,
                                   name="teb")
                    nc.vector.tensor_copy(teb[:], av0[0:DH + 1, :])
                    nc.gpsimd.dma_start(osb[0:DH, tp, :], teb[0:DH, :])
                    nc.sync.dma_start(dscr[2 * tp, :], teb[DH:DH + 1, :])
                    tob = asb.tile([DH + 1, N], bf16, tag="tmpo", bufs=3,
                                   name="tob")
                    nc.vector.tensor_copy(tob[:], av1[0:DH + 1, :])
                    nc.gpsimd.dma_start(osb[DH:P, tp, :], tob[0:DH, :])
                    nc.sync.dma_start(dscr[2 * tp + 1, :], tob[DH:DH + 1, :])
                    # normalize: Rt = 1/denoms broadcast across partitions
                    Rt = asb.tile([P, N], bf16, tag="Rt", bufs=2, name="Rt")
                    for hh in range(2):
                        srcb = bass.AP(
                            tensor=dscr.tensor,
                            offset=dscr.offset + (2 * tp + hh) * N,
                            ap=[[0, DH], [1, N]])
                        nc.gpsimd.dma_start(Rt[hh * DH:(hh + 1) * DH, :], srcb)
                    with nc.allow_low_precision(reason="softmax denom"):
                        nc.vector.reciprocal(Rt[:], Rt[:])
                    nc.gpsimd.tensor_mul(osb[:, tp, :], osb[:, tp, :], Rt[:])

                for it in range(NT):
                    v_tile(it)
                av0 = aps.tile([DH + 1, N], f32, tag="av", bufs=2, name="av0")
                av1 = aps.tile([DH + 1, N], f32, tag="av", bufs=2, name="av1")
                for jp in range(NP):
                    attn_jp(0, jp, av0, av1)
                finish_pair(0, av0, av1)
                for tp in range(1, CT):
                    av0 = aps.tile([DH + 1, N], f32, tag="av", bufs=2,
                                   name="av0")
                    av1 = aps.tile([DH + 1, N], f32, tag="av", bufs=2,
                                   name="av1")
                    for jp in range(NP):
                        attn_jp(tp, jp, av0, av1)
                    finish_pair(tp, av0, av1)
                aps_cm.__exit__(None, None, None)

                # ======== proj (DoubleRow fp8) + residual ========
                with (
                    tc.tile_pool(name="pj_ps", bufs=1, space="PSUM") as pps,
                ):
                    for mt in range(CT):
                        pss = [pps.tile([P, 512], f32, tag="ps", bufs=6,
                                        name="ps") for _ in range(2)]
                        for h in range(2):
                            nc.tensor.matmul(
                                pss[h][:], bprojT_sb[:, mt * P:(mt + 1) * P],
                                ones_row[:, bass.ts(h, 512)],
                                start=True, stop=False)
                        for kp in range(KP):
                            for h in range(2):
                                sl = bass.ts(h, 512)
                                nc.tensor.matmul(
                                    pss[h][:],
                                    wp_sb[:, 2 * kp:2 * kp + 2,
                                          mt * P:(mt + 1) * P],
                                    osb[:, 2 * kp:2 * kp + 2, sl],
                                    start=False, stop=(kp == KP - 1),
                                    perf_mode=DR)
                        for h in range(2):
                            sl = bass.ts(h, 512)
                            nc.vector.scalar_tensor_tensor(
                                out=xsb[:, mt, sl], in0=pss[h][:],
                                scalar=1.0 / WS, op0=ALU.mult,
                                in1=xsb[:, mt, sl], op1=ALU.add)

            # ======== LN2 + MLP (DoubleRow fp8) ========
            with tc.tile_pool(name="mlp_sb", bufs=1) as mw:
                h2 = hpool.tile([P, CT, N], fp8, tag="h")
                layer_norm_T(xsb, h2, g2_sb, b2_sb)
                h3sb = mw.tile([P, HT, N], fp8, name="h3sb")
                with tc.tile_pool(name="fc1_ps", bufs=1, space="PSUM") as f1p:
                    for ct in range(HT):
                        w1m = w1_sb[:, ct, :].rearrange("p (c q) -> p c q", q=P)
                        h3ps = f1p.tile([P, N], f32, tag="h3ps", bufs=3,
                                        name="h3ps")
                        for kp in range(KP):
                            for h in range(2):
                                sl = bass.ts(h, 512)
                                nc.tensor.matmul(
                                    h3ps[:, sl],
                                    w1m[:, 2 * kp:2 * kp + 2, :],
                                    h2[:, 2 * kp:2 * kp + 2, sl],
                                    start=(kp == 0), stop=(kp == KP - 1),
                                    perf_mode=DR)
                        nc.scalar.activation(
                            h3sb[:, ct, :], h3ps[:], AF.Gelu,
                            bias=bfc1_sb[:, ct:ct + 1], scale=DS)
                with tc.tile_pool(name="fc2_ps", bufs=1, space="PSUM") as f2p:
                    for g in range(2):  # output groups: mt 0-2, 3-5
                        f2ps = [f2p.tile([P, N], f32, tag=f"f2_{i}", bufs=1,
                                         name=f"f2ps{i}") for i in range(3)]
                        for i in range(3):
                            for h in range(2):
                                sl = bass.ts(h, 512)
                                nc.tensor.matmul(
                                    f2ps[i][:, sl],
                                    bfc2T_sb[:, (g * 3 + i) * P:
                                             (g * 3 + i + 1) * P],
                                    ones_row[:, sl], start=True, stop=False)
                        for kp in range(HP):
                            for i in range(3):
                                for h in range(2):
                                    sl = bass.ts(h, 512)
                                    nc.tensor.matmul(
                                        f2ps[i][:, sl],
                                        w2v[:, 2 * kp:2 * kp + 2, g,
                                            i * P:(i + 1) * P],
                                        h3sb[:, 2 * kp:2 * kp + 2, sl],
                                        start=False, stop=(kp == HP - 1),
                                        perf_mode=DR)
                        for i in range(3):
                            mt = g * 3 + i
                            nc.vector.scalar_tensor_tensor(
                                out=xsb[:, mt, :], in0=f2ps[i][:],
                                scalar=1.0 / WS, op0=ALU.mult,
                                in1=xsb[:, mt, :], op1=ALU.add)
                            if _rep == reps - 1:
                                nc.sync.dma_start(
                                    _t6(outT)[:, mt, :], xsb[:, mt, :])


_NC_CACHE = None


def _get_nc():
    global _NC_CACHE
    if _NC_CACHE is None:
        _NC_CACHE = build_nc()
    return _NC_CACHE


WS = 64.0     # weight scale into fp8
HS = 16.0     # LN-output (h) scale into fp8
DS = 1.0 / (WS * HS)


def _f8(a, scale=WS):
    return np.clip(np.asarray(a, np.float32) * scale, -240.0, 240.0).astype(
        ml_dtypes.float8_e4m3)


def _prep_shared(qkv_w, qkv_b, proj_w, proj_b, fc1_w, fc1_b, fc2_w, fc2_b,
                 ln1_g, ln1_b, ln2_g, ln2_b):
    c = lambda a: np.ascontiguousarray(np.asarray(a, dtype=np.float32))
    return {
        "wqk": _f8(np.ascontiguousarray(np.asarray(qkv_w, np.float32)[:, :2 * DIM].reshape(CT, P, 12, P).transpose(2, 1, 0, 3).reshape(12, P, CT * P))),
        "wv": _f8(np.ascontiguousarray(np.asarray(qkv_w, np.float32)[:, 2 * DIM:].reshape(CT, P, DIM).transpose(1, 0, 2))),
        "wproj": _f8(np.ascontiguousarray(np.asarray(proj_w, np.float32).reshape(CT, P, DIM).transpose(1, 0, 2))),
        "wfc1": _f8(np.ascontiguousarray(np.asarray(fc1_w, np.float32).reshape(CT, P, HT, P).transpose(2, 1, 0, 3).reshape(HT, P, CT * P))),
        "wfc2": _f8(np.ascontiguousarray(np.asarray(fc2_w, np.float32).reshape(HT, P, 2 * 3 * P))),
        "bqk": c(np.asarray(qkv_b)[:2 * DIM].reshape(12, P).T),
        "bprojT": np.asarray(proj_b, np.float32).reshape(1, DIM).astype(ml_dtypes.bfloat16) * np.float32(WS),
        "bfc2T": np.asarray(fc2_b, np.float32).reshape(1, DIM).astype(ml_dtypes.bfloat16) * np.float32(WS),
        "bv": c(np.asarray(qkv_b)[2 * DIM:]),
        "bproj": c(np.asarray(proj_b).reshape(CT, P).T),
        "bfc1": c(np.asarray(fc1_b).reshape(HT, P).T),
        "bfc2": c(np.asarray(fc2_b).reshape(CT, P).T),
        "g1": c(np.asarray(ln1_g).reshape(CT, P).T * HS),
        "b1": c(np.asarray(ln1_b).reshape(CT, P).T * HS),
        "g2": c(np.asarray(ln2_g).reshape(CT, P).T * HS),
        "b2": c(np.asarray(ln2_b).reshape(CT, P).T * HS),
    }


def run(x, shared, **spmd_kwargs):
    nc = _get_nc()
    x = np.asarray(x, dtype=np.float32)
    in_maps = [
        {**shared, "xT": np.ascontiguousarray(x[b].T)} for b in range(B)
    ]
    res = run_bass_kernel_spmd(nc, in_maps, core_ids=list(range(B)), **spmd_kwargs)
    out = np.stack([res.results[b]["outT"].T for b in range(B)])
    return out.astype(np.float32), res


def kernel(x, ln1_g, ln1_b, qkv_w, qkv_b, proj_w, proj_b,
           ln2_g, ln2_b, fc1_w, fc1_b, fc2_w, fc2_b):
    shared = _prep_shared(qkv_w, qkv_b, proj_w, proj_b, fc1_w, fc1_b,
                          fc2_w, fc2_b, ln1_g, ln1_b, ln2_g, ln2_b)
    out, _ = run(x, shared)
    return out
